# revision 41
# baseline (speedup 1.0000x reference)
"""Fused RoBERTa layer (attention + FFN, LoRA merged) on 8 Trainium2 cores.

Sharding: pure data-parallel over batch (16 batches -> 2 per core), no
collectives. LoRA merged into base weights on host; 1/sqrt(hd) folded into
w_q.

v2 layout (vs v1): attention is organized around the ScalarE exp stream
(the hard floor: 64 exps of [128,1024] ~= 71us). Everything else hides
under it:
  - Scores are 2-head row-packed: kT is a single [128, T] tile per head
    pair (even head on partitions 0-63, odd on 64-127); each score matmul
    contracts K=64 via tile_position (0,0)/(64,0) so the two heads' score
    matmuls run CONCURRENTLY in the PE array (no zero-padding waste).
  - Score PSUM tiles rotate through 3 tags (2 banks each) so the next
    tile's matmuls never wait on the current exp (WAR double-buffer+1).
  - ScalarE does exp ONLY during attention. Denominator row copies go to
    vector/gpsimd; transpose evicts are batched [128,2,128] vector ops.
  - Attention starts at ~8us: only Q0/K0 are emitted before it. The rest
    of Q/K (t2=0 then deferred t2=1), all of V, O-proj of batch 0, and
    the LN1 transposes are PE filler slotted between score/AV matmuls.
  - 12 warmup matmuls on garbage data at t=0 flip the HAM clock gate to
    8/8 before real matmuls arrive; initial DMAs split across 4 queues.

fp8 strategy (DoubleRow double-pumping) as v1: QKV / AV / O-proj / FFN-up
run fp8e4m3 DR (weights pre-scaled 2^7, 2^10 for w_q; inverse scales
folded into exp/gelu/evict scales). FFN-down stays bf16 (fp8 breaks the
2e-2 gate; verified by exact numpy emulation of TRN DR semantics).
Attention normalization: V' carries a ones column so AV emits
unnormalized o rows + a denominator row; dens round-trip through DRAM
([H,T] tile) and come back partition-broadcast, one chunk behind, so the
DMA latency is hidden.
"""

import math
import sys

sys.path.insert(0, "/opt/trn_rl_repo")

import numpy as np
import ml_dtypes

import concourse.bacc as bacc
import concourse.bass as bass
import concourse.tile as tile
from concourse import mybir
from concourse.bass_utils import run_bass_kernel_spmd
from concourse.masks import make_identity

BF16 = mybir.dt.bfloat16
FP8 = mybir.dt.float8e4
F32 = mybir.dt.float32
NP_BF16 = np.dtype(ml_dtypes.bfloat16)
NP_FP8 = np.dtype(ml_dtypes.float8_e4m3)

B, S, D, H, HD, F = 16, 512, 1024, 16, 64, 4096
N_CORES = 8
TB = B // N_CORES
T = TB * S

MM_N = 512
P = 128

WSHIFT = 7
WS = float(2.0 ** WSHIFT)
QSHIFT = 10
QS = float(2.0 ** QSHIFT)
EXP_SCALE = float(2.0 ** (-(WSHIFT + QSHIFT)))
ATT_BIAS = -9 * math.log(2.0)


def _ceil_div(a, b):
    return (a + b - 1) // b


def build_program(cfg):
    D_, F_, T_, TB_, H_, HD_ = (cfg["D"], cfg["F"], cfg["T"], cfg["TB"],
                                cfg["H"], cfg["HD"])
    S_ = T_ // TB_
    KD = D_ // P
    KD2 = KD // 2
    KF = F_ // P
    TCH = T_ // P
    NT = _ceil_div(T_, MM_N)
    NTW = min(MM_N, T_)
    ND = _ceil_div(D_, MM_N)
    NDW = min(MM_N, D_)
    SKC = S_ // P
    SKC2 = SKC // 2
    HPC = P // HD_             # heads per 128-partition chunk (=2)
    VW = HD_ + 1               # V' per-head width (ones column)
    VROW = H_ * VW             # V' row width for one key chunk
    UPW = 1024
    UPT = F_ // UPW

    nc = bacc.Bacc("TRN2", target_bir_lowering=False, debug=False,
                   num_devices=N_CORES)

    # ---- DRAM I/O (fp8 tensors pre-swizzled on host: [ntile, 128, W]) ----
    xT8_d = nc.dram_tensor("xT8", [KD2, P, 2 * T_], FP8,
                           kind="ExternalInput")
    xr_d = nc.dram_tensor("xr", [T_, D_], BF16, kind="ExternalInput")
    wq_d = nc.dram_tensor("wq", [KD2, P, 2 * D_], FP8, kind="ExternalInput")
    wk_d = nc.dram_tensor("wk", [KD2, P, 2 * D_], FP8, kind="ExternalInput")
    wv_d = nc.dram_tensor("wv", [KD2, P, 2 * D_], FP8, kind="ExternalInput")
    wo_d = nc.dram_tensor("wo", [KD2, P, 2 * D_], FP8, kind="ExternalInput")
    wup_d = nc.dram_tensor("wup", [KD2 * UPT, P, 2 * UPW], FP8,
                           kind="ExternalInput")
    wdn_d = nc.dram_tensor("wdn", [F_, D_], BF16, kind="ExternalInput")
    bq_d = nc.dram_tensor("bq", [D_], F32, kind="ExternalInput")
    bk_d = nc.dram_tensor("bk", [D_], F32, kind="ExternalInput")
    bup_d = nc.dram_tensor("bup", [F_], F32, kind="ExternalInput")
    mask_d = nc.dram_tensor("maskT", [TB_, S_], F32, kind="ExternalInput")
    bv_d = nc.dram_tensor("bv", [D_], F32, kind="ExternalInput")
    bo_d = nc.dram_tensor("bo", [D_], F32, kind="ExternalInput")
    bdn_d = nc.dram_tensor("bdn", [D_], F32, kind="ExternalInput")
    g1_d = nc.dram_tensor("g1", [D_], F32, kind="ExternalInput")
    b1_d = nc.dram_tensor("b1", [D_], F32, kind="ExternalInput")
    g2_d = nc.dram_tensor("g2", [D_], F32, kind="ExternalInput")
    b2_d = nc.dram_tensor("b2", [D_], F32, kind="ExternalInput")
    out_d = nc.dram_tensor("out", [T_, D_], F32, kind="ExternalOutput")

    DR = mybir.MatmulPerfMode.DoubleRow

    with tile.TileContext(nc) as tc, \
         tc.tile_pool(name="consts", bufs=1) as consts, \
         tc.tile_pool(name="slab", bufs=1) as slab, \
         tc.tile_pool(name="pall", bufs=1, space="PSUM") as pall, \
         tc.tile_pool(name="work", bufs=2) as work, \
         tc.tile_pool(name="xrp", bufs=2) as xrp, \
         tc.tile_pool(name="attnp", bufs=1) as attnp, \
         tc.tile_pool(name="attn2", bufs=4) as attn2, \
         tc.tile_pool(name="rbp", bufs=1) as rbp, \
         tc.tile_pool(name="statp", bufs=4) as statp, \
         tc.tile_pool(name="outp", bufs=2) as outp, \
         tc.tile_pool(name="dramp", bufs=2, space="DRAM") as dramp:

        dma = nc.sync          # bulk loads
        dma2 = nc.gpsimd       # latency-bound small DMAs
        dma3 = nc.scalar       # second bulk queue (cold start)

        def slot(tag, width, dtype):
            return slab.tile([P, width], dtype, tag=tag, name=f"t_{tag}")

        def pair(ap_2d, i2):
            return ap_2d.rearrange("p (i w) -> p i w", i=2) if i2 is None \
                else ap_2d.rearrange("p (i w) -> p i w", i=2)[:, :, i2]

        # ---- PSUM tags ----
        # sc0/sc1/sc2: rotating 2-bank score tiles (also reused by FFN
        # up/down accumulators after attention). aux: 1-bank tiles shared
        # by AV, projection fills and transposes (2 bufs).
        sc_ctr = [0]

        def sc_tile(width=2 * MM_N, dtype=F32):
            t = pall.tile([P, width], dtype, tag=f"sc{sc_ctr[0] % 3}",
                          name="ps_sc", padded_shape=[P, 2 * MM_N])
            sc_ctr[0] += 1
            return t

        def aux_tile(width=MM_N, dtype=F32):
            return pall.tile([P, width], dtype, tag="aux", bufs=2,
                             name="ps_aux", padded_shape=[P, MM_N])

        # ---- warmup: flip the HAM clock gate before real matmuls ----
        warm_sb = slot("g0", T_, BF16)
        nc.vector.memset(warm_sb[:, 0:MM_N], 0.0)
        for wi in range(12):
            wp = aux_tile()
            nc.tensor.matmul(wp, lhsT=warm_sb[:, 0:P], rhs=warm_sb[:, 0:MM_N],
                             start=True, stop=True)

        # ---- cold-start DMAs: xT8 + wq + wk split across 4 queues ----
        xT8_sb = [slot(f"xT8{c2}", 2 * T_, FP8) for c2 in range(KD2)]
        w_sb = {nm: [slot(f"w{nm}{c2}", 2 * D_, FP8) for c2 in range(KD2)]
                for nm in ("q", "k", "v")}
        # cold loads go ONLY on sync+gpsimd: a dma_start blocks its issuing
        # engine until ring space frees, and ScalarE must be free to start
        # the exp stream at ~14us.
        qs = [dma, dma2]
        qi = [0]

        def cold_load(dst, src):
            qs[qi[0] % 2].dma_start(out=dst, in_=src)
            qi[0] += 1

        def wslice(t_or_d, mlo, mhi):
            # column range [mlo*P, mhi*P) of both halves of a K-pair tile
            return t_or_d.rearrange("p (i w) -> p i w", i=2)[
                :, :, mlo * P:mhi * P]

        # order: everything Q0/K0 needs first (xT8 + m=0 slices of wq/wk),
        # then the rest by first-use time
        for c2 in range(KD2):
            cold_load(xT8_sb[c2], xT8_d[c2])
        for nm, dd in (("q", wq_d), ("k", wk_d)):
            for c2 in range(KD2):
                cold_load(wslice(w_sb[nm][c2], 0, 1), wslice(dd[c2], 0, 1))
        for nm, dd in (("q", wq_d), ("k", wk_d)):
            for c2 in range(KD2):
                cold_load(wslice(w_sb[nm][c2], 1, 8), wslice(dd[c2], 1, 8))
        for c2 in range(KD2):
            cold_load(w_sb["v"][c2], wv_d[c2])

        # ---- constants ----
        eps_t = consts.tile([P, 1], F32)
        nc.vector.memset(eps_t, 1e-5)
        attb_t = consts.tile([P, 1], F32)
        nc.vector.memset(attb_t, ATT_BIAS)
        zero_t = consts.tile([P, 1], F32)
        nc.vector.memset(zero_t, 0.0)
        ident = consts.tile([P, P], BF16)
        make_identity(nc, ident)
        if cfg["has_bq"]:
            bq_sb = consts.tile([P, KD], F32)
            dma.dma_start(out=bq_sb,
                          in_=bq_d.ap().rearrange("(m p) -> p m", p=P))
        if cfg["has_bk"]:
            bk_sb = consts.tile([P, KD], F32)
            dma.dma_start(out=bk_sb,
                          in_=bk_d.ap().rearrange("(m p) -> p m", p=P))
        if cfg["has_bup"]:
            bup_sb = consts.tile([P, KF], F32)
            dma3.dma_start(out=bup_sb,
                           in_=bup_d.ap().rearrange("(m p) -> p m", p=P))
        if cfg["has_mask"]:
            mask_sb = consts.tile([P, TB_ * SKC], F32)
            dma3.dma_start(out=mask_sb,
                           in_=mask_d.ap().rearrange("b (kc p) -> p (b kc)",
                                                     p=P))
            mask2_sb = consts.tile([P, TB_ * SKC], F32)
            nc.vector.tensor_scalar_add(out=mask2_sb, in0=mask_sb,
                                        scalar1=ATT_BIAS)

        def bcast_row(dram_vec, n):
            t = consts.tile([P, n], F32, name=f"bc_{dram_vec.name}")
            dma3.dma_start(out=t,
                           in_=dram_vec.ap().unsqueeze(0).to_broadcast([P, n]))
            return t

        bv_bc = bcast_row(bv_d, D_) if cfg["has_bv"] else None
        bo_bc = bcast_row(bo_d, D_) if cfg["has_bo"] else None
        bdn_bc = bcast_row(bdn_d, D_) if cfg["has_bdn"] else None
        g1_bc = bcast_row(g1_d, D_) if cfg["has_n1"] else None
        b1_bc = bcast_row(b1_d, D_) if cfg["has_n1"] else None
        g2_bc = bcast_row(g2_d, D_) if cfg["has_n2"] else None
        b2_bc = bcast_row(b2_d, D_) if cfg["has_n2"] else None

        qT_sb = [slot(f"qT{c}", T_, BF16) for c in range(KD)]
        kT_sb = [slot(f"kT{c}", T_, BF16) for c in range(KD)]
        Vp8_sb = [slot(f"Vp{c}", 2 * VROW, FP8) for c in range(TCH // 2)]

        HB = P // 2

        # ---- QKV projections (fp8 DoubleRow) ----
        def qk_proj(nm, m, t2):
            has_b = cfg["has_bq"] if nm == "q" else cfg["has_bk"]
            bias = (bq_sb if nm == "q" else bk_sb) if has_b else None
            pt = aux_tile()
            for c2 in range(KD2):
                nc.tensor.matmul(
                    pt[:, :NTW],
                    lhsT=pair(w_sb[nm][c2], slice(m * P, (m + 1) * P)),
                    rhs=pair(xT8_sb[c2], slice(t2 * MM_N, t2 * MM_N + NTW)),
                    start=(c2 == 0), stop=(c2 == KD2 - 1),
                    perf_mode=DR)
            sl = slice(t2 * MM_N, t2 * MM_N + NTW)
            dst = (qT_sb if nm == "q" else kT_sb)[m]
            if has_b:
                nc.vector.tensor_scalar_add(out=dst[:, sl], in0=pt[:, :NTW],
                                            scalar1=bias[:, m:m + 1])
            elif nm == "k":
                # ScalarE absorbs K evicts in its exp-stall slack ('copy'
                # is in every act table, so no table thrash)
                nc.scalar.copy(out=dst[:, sl], in_=pt[:, :NTW])
            else:
                nc.vector.tensor_copy(out=dst[:, sl], in_=pt[:, :NTW])

        # V token-major into V' ([v(64), 1] per head; 2^-7 scale on evict)
        def v_proj_tr(tr):
            vdst = Vp8_sb[tr // 2][:, (tr % 2) * VROW:(tr % 2 + 1) * VROW]
            vd3 = vdst.rearrange("p (h c) -> p h c", c=VW)
            for n2 in range(ND):
                pt = aux_tile()
                for c2 in range(KD2):
                    nc.tensor.matmul(
                        pt[:, :NDW],
                        lhsT=pair(xT8_sb[c2], slice(tr * P, (tr + 1) * P)),
                        rhs=pair(w_sb["v"][c2],
                                 slice(n2 * MM_N, n2 * MM_N + NDW)),
                        start=(c2 == 0), stop=(c2 == KD2 - 1),
                        perf_mode=DR)
                hpn = NDW // HD_   # heads per N tile
                src = pt[:, :NDW].rearrange("p (h c) -> p h c", c=HD_)
                if cfg["has_bv"]:
                    tmp = work.tile([P, NDW], F32, tag="vtmp", name="vtmp")
                    nc.vector.tensor_add(
                        out=tmp, in0=pt[:, :NDW],
                        in1=bv_bc[:, n2 * MM_N:n2 * MM_N + NDW])
                    src = tmp.rearrange("p (h c) -> p h c", c=HD_)
                nc.vector.tensor_scalar_mul(
                    out=vd3[:, n2 * hpn:(n2 + 1) * hpn, 0:HD_], in0=src,
                    scalar1=1.0 / WS)
            nc.vector.memset(vd3[:, :, HD_:VW], 1.0)  # ones cols

        # ---- attention machinery ----
        # wo loads follow wv on the bulk queues (needed mid-b1 for fills)
        wo_sb = []
        for c2 in range(KD2):
            t = slot(f"wo{c2}", 2 * D_, FP8)
            cold_load(t, wo_d[c2])
            wo_sb.append(t)
        oT8_sb = [slot(f"oT{c2}", 2 * T_, FP8) for c2 in range(KD2)]
        oTu_sb = [slot(f"oTu{hc}", T_, BF16) for hc in range(KD)]
        den_d = dramp.tile([H_, T_], F32, tag="den_d", name="den_d")
        rb_sb = {}

        def at_tile():
            return attnp.tile([P, 2 * S_], FP8, tag="attnT", bufs=10 + 2,
                              name="attnT")

        def attn_scores(b, hc):
            """Row-packed scores for head pair hc: 4 psum tiles
            (E-kc01, O-kc01, E-kc23, O-kc23), one exp each -> 4 at tiles
            (kc-paired fp8, ready for DR AV)."""
            ats = []
            for half in range(2):          # kc01 / kc23
                pts = [sc_tile(), sc_tile()]   # [even-head, odd-head]
                for k2 in range(2):
                    kc = 2 * half + k2
                    for par in range(HPC):
                        # K=64 at base partition 0/64: the row group
                        # auto-derives, so the two heads' matmuls share
                        # the array concurrently
                        nc.tensor.matmul(
                            pts[par][:, k2 * S_:(k2 + 1) * S_],
                            lhsT=kT_sb[hc][par * HD_:(par + 1) * HD_,
                                           b * S_ + kc * P:
                                           b * S_ + (kc + 1) * P],
                            rhs=qT_sb[hc][par * HD_:(par + 1) * HD_,
                                          b * S_:(b + 1) * S_],
                            start=True, stop=True)
                for par in range(HPC):
                    at = at_tile()
                    if cfg["has_mask"]:
                        for k2 in range(2):
                            kc = 2 * half + k2
                            nc.scalar.activation(
                                out=at[:, k2 * S_:(k2 + 1) * S_],
                                in_=pts[par][:, k2 * S_:(k2 + 1) * S_],
                                func=mybir.ActivationFunctionType.Exp,
                                bias=mask2_sb[:, b * SKC + kc:
                                              b * SKC + kc + 1],
                                scale=EXP_SCALE)
                    else:
                        nc.scalar.activation(
                            out=at, in_=pts[par][:, 0:2 * S_],
                            func=mybir.ActivationFunctionType.Exp,
                            bias=attb_t, scale=EXP_SCALE)
                    ats.append(at)
            # ats = [E-kc01, O-kc01, E-kc23, O-kc23]
            return ats

        def attn_av(b, hc, ats):
            for par in range(HPC):
                h = hc * HPC + par
                pv = aux_tile()
                for half in range(2):
                    nc.tensor.matmul(
                        pv[0:VW, :S_],
                        lhsT=pair(Vp8_sb[b * SKC2 + half],
                                  slice(h * VW, (h + 1) * VW)),
                        rhs=pair(ats[2 * half + par], None),
                        start=(half == 0), stop=(half == 1),
                        perf_mode=DR)
                ho = par * HD_
                nc.vector.tensor_copy(
                    out=oTu_sb[hc][ho:ho + HD_, b * S_:(b + 1) * S_],
                    in_=pv[0:HD_, :S_])
                rs = attn2.tile([1, S_], F32, tag="rs", bufs=3, name="rs")
                nc.vector.tensor_copy(out=rs, in_=pv[HD_:VW, :S_])
                dma2.dma_start(out=den_d[h:h + 1, b * S_:(b + 1) * S_],
                               in_=rs)

        def rb_load(b, hc):
            # broadcast this chunk's denominators back from DRAM
            sl = slice(b * S_, (b + 1) * S_)
            rb = rbp.tile([P, S_], F32, tag=f"rb{hc % 4}", name="rb")
            rb_sb[hc] = rb
            for h2 in range(HPC):
                dma2.dma_start(
                    out=rb[h2 * HD_:(h2 + 1) * HD_, :],
                    in_=den_d[HPC * hc + h2:HPC * hc + h2 + 1, sl]
                    .to_broadcast([HD_, S_]))

        def recip_mul(b, hc):
            # reciprocal + normalize one feature chunk: oT8 = oTu / den.
            # Runs one chunk behind rb_load so the DMA latency is hidden.
            sl = slice(b * S_, (b + 1) * S_)
            rb = rb_sb[hc]
            nc.vector.reciprocal_approx_fast(out=rb, in_=rb)
            nc.vector.tensor_mul(
                out=oT8_sb[hc // 2][:, (hc % 2) * T_ + b * S_:
                                    (hc % 2) * T_ + (b + 1) * S_],
                in0=oTu_sb[hc][:, sl], in1=rb)

        # O-proj machinery; LN1 computes rstd with a vector-side Newton
        # rsqrt (seed 2^-7: the LN1 input is 2^7-scaled, so var ~= 2^14)
        # so no ScalarE act-table switch ever interrupts the exp stream.
        xm_bf = {}
        xmT8_sb = [slot(f"xmT{c2}", 2 * T_, FP8) for c2 in range(KD2)]

        def newton_rstd(v_col, eng):
            # 1/sqrt(v) for v ~ 2^14 * [0.8, 2.0]; 3 iterations to fp32-ish
            y = statp.tile([P, 1], F32, tag="nwy", name="nwy")
            t = statp.tile([P, 1], F32, tag="nwt", name="nwt")
            eng.memset(y, 2.0 ** -7)
            for _ in range(3):
                eng.tensor_mul(out=t, in0=y, in1=y)
                eng.tensor_mul(out=t, in0=t, in1=v_col)
                eng.tensor_scalar(
                    out=t, in0=t, scalar1=-0.5, scalar2=1.5,
                    op0=mybir.AluOpType.mult, op1=mybir.AluOpType.add)
                eng.tensor_mul(out=y, in0=y, in1=t)
            return y

        def ln1_tr(tr):
            # in-place LayerNorm on the bf16 x_medium tile (vector-only;
            # gpsimd bulk elementwise is ~17x slower than DVE)
            xm = xm_bf[tr]
            bw = min(512, D_)
            nsub = _ceil_div(D_, bw)
            st = statp.tile([P, nsub, 6], F32, tag="bnst", name="bnst")
            for i in range(nsub):
                nc.vector.bn_stats(out=st[:, i, :],
                                   in_=xm[:, i * bw:(i + 1) * bw])
            mv = statp.tile([P, 2], F32, tag="bnmv", name="bnmv")
            nc.vector.bn_aggr(out=mv, in_=st)
            rstd = newton_rstd(mv[:, 1:2], nc.vector)
            if cfg["has_n1"]:
                tmp = statp.tile([P, D_], F32, tag="lntmp", name="lntmp")
                nc.vector.tensor_scalar(
                    out=tmp, in0=xm, scalar1=mv[:, 0:1], scalar2=rstd,
                    op0=mybir.AluOpType.subtract, op1=mybir.AluOpType.mult)
                nc.vector.tensor_mul(out=tmp, in0=tmp, in1=g1_bc)
                nc.vector.tensor_add(out=xm, in0=tmp, in1=b1_bc)
            else:
                nc.vector.tensor_scalar(
                    out=xm, in0=xm, scalar1=mv[:, 0:1], scalar2=rstd,
                    op0=mybir.AluOpType.subtract, op1=mybir.AluOpType.mult)

        xr_tiles = {}

        def xr_load(tr):
            xt = xrp.tile([P, D_], BF16, tag="xrt", name="xrt")
            dma2.dma_start(out=xt, in_=xr_d[tr * P:(tr + 1) * P, :])
            xr_tiles[tr] = xt

        def o_mm_tr(tr):
            # O-projection matmuls + residual add -> bf16 xm (pre-LN)
            xt = xr_tiles[tr]
            xm = slot(f"qT{tr}", D_, BF16)   # reuse qT slot (scores done)
            xm_bf[tr] = xm
            for n2 in range(ND):
                pt = aux_tile()
                for c2 in range(KD2):
                    nc.tensor.matmul(
                        pt[:, :NDW],
                        lhsT=pair(oT8_sb[c2], slice(tr * P, (tr + 1) * P)),
                        rhs=pair(wo_sb[c2],
                                 slice(n2 * MM_N, n2 * MM_N + NDW)),
                        start=(c2 == 0), stop=(c2 == KD2 - 1),
                        perf_mode=DR)
                nc.vector.tensor_add(out=xm[:, n2 * MM_N:n2 * MM_N + NDW],
                                     in0=pt[:, :NDW],
                                     in1=xt[:, n2 * MM_N:n2 * MM_N + NDW])
                if cfg["has_bo"]:
                    nc.vector.tensor_add(
                        out=xm[:, n2 * MM_N:n2 * MM_N + NDW],
                        in0=xm[:, n2 * MM_N:n2 * MM_N + NDW],
                        in1=bo_bc[:, n2 * MM_N:n2 * MM_N + NDW])

        def o_proj_tr(tr):
            o_mm_tr(tr)
            ln1_tr(tr)

        def transpose_tr(tr):
            # PE transposes, evicted 2-at-a-time with a 3D [128,2,128] AP
            # (DVE only: gpsimd has no PSUM port)
            for c2 in range(KD2):
                pt = pall.tile([P, 2 * P], BF16, tag="aux", bufs=2,
                               name="ps_t", padded_shape=[P, MM_N])
                for j in range(2):
                    c = 2 * c2 + j
                    nc.tensor.transpose(pt[:, j * P:(j + 1) * P],
                                        xm_bf[tr][:, c * P:(c + 1) * P],
                                        ident)
                dst = xmT8_sb[c2].rearrange(
                    "p (i w) -> p i w", i=2)[:, :, tr * P:(tr + 1) * P]
                nc.vector.tensor_copy(out=dst,
                                      in_=pt.rearrange("p (i w) -> p i w",
                                                       i=2))

        # ---- FFN up helpers (t2-split halves) ----
        wup_sb = {}

        def wup_load(i, tag, cold=False):
            t = slot(tag, 2 * UPW, FP8)
            if cold:
                cold_load(t, wup_d[i])
            else:
                dma.dma_start(out=t, in_=wup_d[i])
            wup_sb[i] = t

        # the fm<16 half of wup goes into the idle dn tags NOW (trickles
        # in during b0 attention) so FFN up can start the moment the
        # attention loop ends
        for c2 in range(KD2):
            wup_load(c2 * UPT + 0, f"dn{c2}", cold=True)
        for c2 in range(KD2):
            wup_load(c2 * UPT + 1, f"dn{4 + c2}", cold=True)

        def wup_lhsT(c2, fm):
            i = c2 * UPT + (fm * P) // UPW
            o = (fm * P) % UPW
            return pair(wup_sb[i], slice(o, o + P))

        gT_sb = {}
        # tag order matters: oT tags free only after o_proj_tr(7), which is
        # woven at up-t2=0 fm==23 -> oT tags must serve fm>=28 only
        g_tags = ([f"g{c}" for c in range(KF - KD - 2 * KD2)]
                  + [f"wv{c2}" for c2 in range(KD2)]
                  + [f"oTu{hc}" for hc in range(KD)]
                  + [f"oT{c2}" for c2 in range(KD2)])

        def up_half(fm, t2):
            pt = aux_tile()
            for c2 in range(KD2):
                nc.tensor.matmul(
                    pt[:, :NTW],
                    lhsT=wup_lhsT(c2, fm),
                    rhs=pair(xmT8_sb[c2],
                             slice(t2 * MM_N, t2 * MM_N + NTW)),
                    start=(c2 == 0), stop=(c2 == KD2 - 1),
                    perf_mode=DR)
            if fm not in gT_sb:
                gT_sb[fm] = slot(g_tags[fm], T_, BF16)
            nc.scalar.activation(
                out=gT_sb[fm][:, t2 * MM_N:t2 * MM_N + NTW],
                in_=pt[:, :NTW],
                func=mybir.ActivationFunctionType.Gelu,
                bias=(bup_sb[:, fm:fm + 1] if cfg["has_bup"] else zero_t),
                scale=1.0 / WS)

        # ---- emit: Q0/K0 then the exp-stream-driven attention loop ----
        qk_proj("q", 0, 0)
        qk_proj("k", 0, 0)

        # fills per (b, pair-index): list of thunks. Constraints:
        #  - v(0..3) emitted by slot (0,1) (first AV dequeues at (0,2));
        #    v(4..7) by slot (1,1).
        #  - qk(m,0) by slot (0,m-1); qk(m,1) by slot (1,m-1) (or in b0).
        def F_qk(m, t2):
            return lambda: (qk_proj("q", m, t2), qk_proj("k", m, t2))

        def F_v(tr):
            return lambda: v_proj_tr(tr)

        fills = {
            (0, 0): [F_qk(1, 0), F_v(0), F_v(1)],
            (0, 1): [F_qk(2, 0), F_v(2), F_v(3)],
            (0, 2): [F_qk(3, 0)], (0, 3): [F_qk(4, 0)],
            (0, 4): [F_qk(5, 0)], (0, 5): [F_qk(6, 0)],
            (0, 6): [F_qk(7, 0), F_qk(0, 1)],
            (0, 7): [F_qk(1, 1), F_v(4)],
            (1, 0): [F_v(5), F_qk(2, 1)],
            (1, 1): [F_v(6), F_v(7), F_qk(3, 1)],
            (1, 2): [F_qk(4, 1)],
            (1, 3): [F_qk(5, 1)],
            (1, 4): [F_qk(6, 1)],
            (1, 5): [F_qk(7, 1)],
        }

        avq = []   # 2-deep AV lag so V fills land before the first AV
        rmq = []   # recip_mul runs one chunk behind rb_load

        def av_dequeue():
            pb, phc, ats = avq.pop(0)
            attn_av(pb, phc, ats)
            rb_load(pb, phc)
            rmq.append((pb, phc))
            if len(rmq) >= 2:
                recip_mul(*rmq.pop(0))

        for b in range(TB_):
            for hc in range(KD):
                ats = attn_scores(b, hc)
                avq.append((b, hc, ats))
                if len(avq) > 2:
                    av_dequeue()
                for f in fills.get((b, hc), []):
                    f()
        while avq:
            av_dequeue()
        while rmq:
            recip_mul(*rmq.pop(0))

        # ---- post-attention ----
        # wup fm>=16 half: j=2 blocks into wq tags (free after the
        # deferred QK fills), j=3 into wk tags.
        for tr in range(TCH):
            xr_load(tr)
        for c2 in range(KD2):
            wup_load(c2 * UPT + 2, f"wq{c2}")
        for c2 in range(KD2):
            wup_load(c2 * UPT + 3, f"wk{c2}")
        # O-proj/LN1/transpose of trs 0-3 unlock up-t2=0 (its xmT8 column
        # slice only spans batch-0 tokens)
        o_proj_tr(0)
        o_proj_tr(1)
        transpose_tr(0)
        o_proj_tr(2)
        transpose_tr(1)
        o_proj_tr(3)
        transpose_tr(2)
        transpose_tr(3)

        # FFN down weight loads issued early so the DMA hides under up
        # wo tags free only after o_proj_tr(7): keep them last so the sync
        # queue isn't blocked mid-stream waiting on the weave
        dn_tags = ([f"Vp{c}" for c in range(TCH // 2)]
                   + [f"xT8{c2}" for c2 in range(KD2)]
                   + [f"kT{c}" for c in range(KD)]
                   + [f"dn{i}" for i in range(8)]
                   + [f"wo{c2}" for c2 in range(KD2)])
        wdn_sb = []
        for fc in range(KF):
            if fc < len(dn_tags):
                t = slot(dn_tags[fc], D_, BF16)
            else:
                t = rbp.tile([P, D_], BF16, tag=f"rb{fc - len(dn_tags)}",
                             name="t_wdn_rb")
            # sync queue only: scalar must stay free for the gelu stream
            dma.dma_start(out=t, in_=wdn_d[fc * P:(fc + 1) * P, :])
            wdn_sb.append(t)

        # up-t2=0 matmuls hide the vector-bound O/LN1 chains of trs 4-7
        weave = {2: lambda: o_proj_tr(4), 7: lambda: transpose_tr(4),
                 9: lambda: o_proj_tr(5), 14: lambda: transpose_tr(5),
                 16: lambda: o_proj_tr(6), 21: lambda: transpose_tr(6),
                 23: lambda: o_proj_tr(7), 28: lambda: transpose_tr(7)}
        for fm in range(KF):
            up_half(fm, 0)
            if fm in weave:
                weave[fm]()
        for fm in range(KF):
            up_half(fm, 1)

        # ---- FFN down (bf16) + residual + LN2 -> out, incremental ----
        def layer_norm_apply(src_t, dst, mv, rstd, g_bc, b_bc):
            if g_bc is None:
                nc.vector.tensor_scalar(
                    out=dst, in0=src_t, scalar1=mv[:, 0:1], scalar2=rstd,
                    op0=mybir.AluOpType.subtract, op1=mybir.AluOpType.mult)
            else:
                tmp = statp.tile([P, D_], F32, tag="lntmp", name="lntmp")
                nc.vector.tensor_scalar(
                    out=tmp, in0=src_t, scalar1=mv[:, 0:1], scalar2=rstd,
                    op0=mybir.AluOpType.subtract, op1=mybir.AluOpType.mult)
                nc.vector.tensor_mul(out=tmp, in0=tmp, in1=g_bc)
                nc.vector.tensor_add(out=dst, in0=tmp, in1=b_bc)

        for tr in range(TCH):
            dsb = work.tile([P, D_], F32, tag="acc", name="dsb")
            st = statp.tile([P, ND, 6], F32, tag="bnst", name="bnst")
            pt = sc_tile()
            for n2 in range(ND):
                for fc in range(KF):
                    nc.tensor.matmul(
                        pt[:, n2 * MM_N:n2 * MM_N + NDW],
                        lhsT=gT_sb[fc][:, tr * P:(tr + 1) * P],
                        rhs=wdn_sb[fc][:, n2 * MM_N:n2 * MM_N + NDW],
                        start=(fc == 0), stop=(fc == KF - 1))
                # evict+add+stats per half so only the last half's chain
                # is exposed after the final matmul
                sl = slice(n2 * MM_N, n2 * MM_N + NDW)
                nc.vector.tensor_add(out=dsb[:, sl], in0=pt[:, sl],
                                     in1=xm_bf[tr][:, sl])
                if cfg["has_bdn"]:
                    nc.vector.tensor_add(out=dsb[:, sl], in0=dsb[:, sl],
                                         in1=bdn_bc[:, sl])
                nc.vector.bn_stats(out=st[:, n2, :], in_=dsb[:, sl])
            mv = statp.tile([P, 2], F32, tag="bnmv", name="bnmv")
            nc.vector.bn_aggr(out=mv, in_=st)
            rstd = statp.tile([P, 1], F32, tag="rstd", name="rstd")
            nc.scalar.activation(out=rstd, in_=mv[:, 1:2],
                                 func=mybir.ActivationFunctionType.Sqrt,
                                 bias=eps_t, scale=1.0)
            nc.vector.reciprocal(out=rstd, in_=rstd)
            ot = outp.tile([P, D_], F32, tag="ot", name="ot")
            layer_norm_apply(dsb, ot, mv, rstd,
                             g2_bc if cfg["has_n2"] else None,
                             b2_bc if cfg["has_n2"] else None)
            if tr < TCH - 1:
                dma.dma_start(out=out_d[tr * P:(tr + 1) * P, :], in_=ot)
            else:
                # last chunk is latency-exposed: split across HW queues
                qw = D_ // 4
                engs = (nc.sync, nc.scalar, nc.sync, nc.scalar)
                for qi, eng in enumerate(engs):
                    eng.dma_start(
                        out=out_d[tr * P:(tr + 1) * P,
                                  qi * qw:(qi + 1) * qw],
                        in_=ot[:, qi * qw:(qi + 1) * qw])

    nc.finalize()
    return nc


_PROGRAM_CACHE = {}


def _get_program(cfg_key, cfg):
    if cfg_key not in _PROGRAM_CACHE:
        _PROGRAM_CACHE[cfg_key] = build_program(cfg)
    return _PROGRAM_CACHE[cfg_key]


def _swz(w, npairs, width):
    """[rows, cols] -> [npairs, 128, 2*cols] K-paired contiguous."""
    return np.ascontiguousarray(
        w.reshape(npairs, 2, P, width).transpose(0, 2, 1, 3)
        .reshape(npairs, P, 2 * width))


def make_in_maps(inputs):
    f32 = np.float32
    x = np.asarray(inputs["x"], f32)
    scale = 1.0 / np.sqrt(float(inputs["head_dim"]))

    def merged(w, a, b):
        return (np.asarray(w, f32)
                + np.asarray(a, f32) @ np.asarray(b, f32))

    KD2 = D // P // 2
    wq = _swz((merged(inputs["w_q"], inputs["w_q_lora_a"],
                      inputs["w_q_lora_b"]) * (scale * QS)).astype(NP_FP8),
              KD2, D)
    wk = _swz((merged(inputs["w_k"], inputs["w_k_lora_a"],
                      inputs["w_k_lora_b"]) * WS).astype(NP_FP8), KD2, D)
    wv = _swz((merged(inputs["w_v"], inputs["w_v_lora_a"],
                      inputs["w_v_lora_b"]) * WS).astype(NP_FP8), KD2, D)
    wo = _swz((merged(inputs["w_o"], inputs["w_o_lora_a"],
                      inputs["w_o_lora_b"]) * WS).astype(NP_FP8), KD2, D)
    wup8 = (merged(inputs["w_up"], inputs["w_up_lora_a"],
                   inputs["w_up_lora_b"]) * WS).astype(NP_FP8)
    UPW = 1024
    UPT = F // UPW
    wup = np.ascontiguousarray(
        wup8.reshape(KD2, 2, P, UPT, UPW).transpose(0, 3, 2, 1, 4)
        .reshape(KD2 * UPT, P, 2 * UPW))
    wdn = merged(inputs["w_down"], inputs["w_down_lora_a"],
                 inputs["w_down_lora_b"]).astype(NP_BF16)
    mask = np.asarray(inputs["attention_mask"], f32)

    common = {
        "wq": wq, "wk": wk, "wv": wv, "wo": wo, "wup": wup, "wdn": wdn,
        "bq": (np.asarray(inputs["b_q"], f32) * (scale * QS)).astype(f32),
        "bk": (np.asarray(inputs["b_k"], f32) * WS).astype(f32),
        "bup": np.asarray(inputs["b_up"], f32),
        "bv": np.asarray(inputs["b_v"], f32),
        "bo": np.asarray(inputs["b_o"], f32),
        "bdn": np.asarray(inputs["b_down"], f32),
        "g1": np.asarray(inputs["norm_weight_1"], f32),
        "b1": np.asarray(inputs["norm_bias_1"], f32),
        "g2": np.asarray(inputs["norm_weight_2"], f32),
        "b2": np.asarray(inputs["norm_bias_2"], f32),
    }
    in_maps = []
    for i in range(N_CORES):
        xc = x[i * TB:(i + 1) * TB].reshape(T, D)
        m = dict(common)
        m["xT8"] = _swz(np.ascontiguousarray(xc.T).astype(NP_FP8), KD2, T)
        m["xr"] = (np.ascontiguousarray(xc) * WS).astype(NP_BF16)
        m["maskT"] = np.ascontiguousarray(mask[i * TB:(i + 1) * TB, 0, 0, :])
        in_maps.append(m)
    return in_maps


def full_cfg(inputs):
    f32 = np.float32
    return {
        "D": D, "F": F, "T": T, "TB": TB, "H": H, "HD": HD,
        "has_bq": bool(np.any(np.asarray(inputs["b_q"], f32))),
        "has_bk": bool(np.any(np.asarray(inputs["b_k"], f32))),
        "has_bup": bool(np.any(np.asarray(inputs["b_up"], f32))),
        "has_mask": bool(np.any(np.asarray(inputs["attention_mask"], f32))),
        "has_bv": bool(np.any(np.asarray(inputs["b_v"], f32))),
        "has_bo": bool(np.any(np.asarray(inputs["b_o"], f32))),
        "has_bdn": bool(np.any(np.asarray(inputs["b_down"], f32))),
        "has_n1": bool(np.any(np.asarray(inputs["norm_weight_1"], f32) != 1.0)
                       or np.any(np.asarray(inputs["norm_bias_1"], f32))),
        "has_n2": bool(np.any(np.asarray(inputs["norm_weight_2"], f32) != 1.0)
                       or np.any(np.asarray(inputs["norm_bias_2"], f32))),
    }


def run_on_hw(inputs, trace=False, tmpdir=None):
    cfg = full_cfg(inputs)
    cfg_key = tuple(sorted((k, v) for k, v in cfg.items()
                           if not isinstance(v, set)))
    nc = _get_program(cfg_key, cfg)
    in_maps = make_in_maps(inputs)
    kw = {}
    if trace:
        kw = {"trace": True, "tmpdir": tmpdir}
    res = run_bass_kernel_spmd(nc, in_maps, core_ids=list(range(N_CORES)),
                               **kw)
    out = np.empty((B, S, D), np.float32)
    for i in range(N_CORES):
        out[i * TB:(i + 1) * TB] = res.results[i]["out"].reshape(TB, S, D)
    return out, res


def kernel(**inputs):
    out, _ = run_on_hw(inputs)
    return out


# revision 42
# speedup vs baseline: 1.0261x; 1.0261x over previous
"""Fused RoBERTa layer (attention + FFN, LoRA merged) on 8 Trainium2 cores.

Sharding: pure data-parallel over batch (16 batches -> 2 per core), no
collectives. LoRA merged into base weights on host; 1/sqrt(hd) folded into
w_q.

v2 layout (vs v1): attention is organized around the ScalarE exp stream
(the hard floor: 64 exps of [128,1024] ~= 71us). Everything else hides
under it:
  - Scores are 2-head row-packed: kT is a single [128, T] tile per head
    pair (even head on partitions 0-63, odd on 64-127); each score matmul
    contracts K=64 via tile_position (0,0)/(64,0) so the two heads' score
    matmuls run CONCURRENTLY in the PE array (no zero-padding waste).
  - Score PSUM tiles rotate through 3 tags (2 banks each) so the next
    tile's matmuls never wait on the current exp (WAR double-buffer+1).
  - ScalarE does exp ONLY during attention. Denominator row copies go to
    vector/gpsimd; transpose evicts are batched [128,2,128] vector ops.
  - Attention starts at ~8us: only Q0/K0 are emitted before it. The rest
    of Q/K (t2=0 then deferred t2=1), all of V, O-proj of batch 0, and
    the LN1 transposes are PE filler slotted between score/AV matmuls.
  - 12 warmup matmuls on garbage data at t=0 flip the HAM clock gate to
    8/8 before real matmuls arrive; initial DMAs split across 4 queues.

fp8 strategy (DoubleRow double-pumping) as v1: QKV / AV / O-proj / FFN-up
run fp8e4m3 DR (weights pre-scaled 2^7, 2^10 for w_q; inverse scales
folded into exp/gelu/evict scales). FFN-down stays bf16 (fp8 breaks the
2e-2 gate; verified by exact numpy emulation of TRN DR semantics).
Attention normalization: V' carries a ones column so AV emits
unnormalized o rows + a denominator row; dens round-trip through DRAM
([H,T] tile) and come back partition-broadcast, one chunk behind, so the
DMA latency is hidden.
"""

import math
import sys

sys.path.insert(0, "/opt/trn_rl_repo")

import numpy as np
import ml_dtypes

import concourse.bacc as bacc
import concourse.bass as bass
import concourse.tile as tile
from concourse import mybir
from concourse.bass_utils import run_bass_kernel_spmd
from concourse.masks import make_identity

BF16 = mybir.dt.bfloat16
FP8 = mybir.dt.float8e4
F32 = mybir.dt.float32
NP_BF16 = np.dtype(ml_dtypes.bfloat16)
NP_FP8 = np.dtype(ml_dtypes.float8_e4m3)

B, S, D, H, HD, F = 16, 512, 1024, 16, 64, 4096
N_CORES = 8
TB = B // N_CORES
T = TB * S

MM_N = 512
P = 128

WSHIFT = 7
WS = float(2.0 ** WSHIFT)
QSHIFT = 10
QS = float(2.0 ** QSHIFT)
EXP_SCALE = float(2.0 ** (-(WSHIFT + QSHIFT)))
ATT_BIAS = -9 * math.log(2.0)


def _ceil_div(a, b):
    return (a + b - 1) // b


def build_program(cfg):
    D_, F_, T_, TB_, H_, HD_ = (cfg["D"], cfg["F"], cfg["T"], cfg["TB"],
                                cfg["H"], cfg["HD"])
    S_ = T_ // TB_
    KD = D_ // P
    KD2 = KD // 2
    KF = F_ // P
    TCH = T_ // P
    NT = _ceil_div(T_, MM_N)
    NTW = min(MM_N, T_)
    ND = _ceil_div(D_, MM_N)
    NDW = min(MM_N, D_)
    SKC = S_ // P
    SKC2 = SKC // 2
    HPC = P // HD_             # heads per 128-partition chunk (=2)
    VW = HD_ + 1               # V' per-head width (ones column)
    VROW = H_ * VW             # V' row width for one key chunk
    UPW = 1024
    UPT = F_ // UPW

    nc = bacc.Bacc("TRN2", target_bir_lowering=False, debug=False,
                   num_devices=N_CORES)

    # ---- DRAM I/O (fp8 tensors pre-swizzled on host: [ntile, 128, W]) ----
    xT8_d = nc.dram_tensor("xT8", [KD2, P, 2 * T_], FP8,
                           kind="ExternalInput")
    xr_d = nc.dram_tensor("xr", [T_, D_], BF16, kind="ExternalInput")
    wq_d = nc.dram_tensor("wq", [KD2, P, 2 * D_], FP8, kind="ExternalInput")
    wk_d = nc.dram_tensor("wk", [KD2, P, 2 * D_], FP8, kind="ExternalInput")
    wv_d = nc.dram_tensor("wv", [KD2, P, 2 * D_], FP8, kind="ExternalInput")
    wo_d = nc.dram_tensor("wo", [KD2, P, 2 * D_], FP8, kind="ExternalInput")
    wup_d = nc.dram_tensor("wup", [KD2 * UPT, P, 2 * UPW], FP8,
                           kind="ExternalInput")
    wdn_d = nc.dram_tensor("wdn", [F_, D_], BF16, kind="ExternalInput")
    bq_d = nc.dram_tensor("bq", [D_], F32, kind="ExternalInput")
    bk_d = nc.dram_tensor("bk", [D_], F32, kind="ExternalInput")
    bup_d = nc.dram_tensor("bup", [F_], F32, kind="ExternalInput")
    mask_d = nc.dram_tensor("maskT", [TB_, S_], F32, kind="ExternalInput")
    bv_d = nc.dram_tensor("bv", [D_], F32, kind="ExternalInput")
    bo_d = nc.dram_tensor("bo", [D_], F32, kind="ExternalInput")
    bdn_d = nc.dram_tensor("bdn", [D_], F32, kind="ExternalInput")
    g1_d = nc.dram_tensor("g1", [D_], F32, kind="ExternalInput")
    b1_d = nc.dram_tensor("b1", [D_], F32, kind="ExternalInput")
    g2_d = nc.dram_tensor("g2", [D_], F32, kind="ExternalInput")
    b2_d = nc.dram_tensor("b2", [D_], F32, kind="ExternalInput")
    out_d = nc.dram_tensor("out", [T_, D_], F32, kind="ExternalOutput")

    DR = mybir.MatmulPerfMode.DoubleRow

    with tile.TileContext(nc) as tc, \
         tc.tile_pool(name="consts", bufs=1) as consts, \
         tc.tile_pool(name="slab", bufs=1) as slab, \
         tc.tile_pool(name="pall", bufs=1, space="PSUM") as pall, \
         tc.tile_pool(name="work", bufs=2) as work, \
         tc.tile_pool(name="xrp", bufs=2) as xrp, \
         tc.tile_pool(name="attnp", bufs=1) as attnp, \
         tc.tile_pool(name="attn2", bufs=4) as attn2, \
         tc.tile_pool(name="rbp", bufs=1) as rbp, \
         tc.tile_pool(name="statp", bufs=4) as statp, \
         tc.tile_pool(name="outp", bufs=2) as outp, \
         tc.tile_pool(name="dramp", bufs=2, space="DRAM") as dramp:

        dma = nc.sync          # bulk loads
        dma2 = nc.gpsimd       # latency-bound small DMAs
        dma3 = nc.scalar       # second bulk queue (cold start)

        def slot(tag, width, dtype):
            return slab.tile([P, width], dtype, tag=tag, name=f"t_{tag}")

        def pair(ap_2d, i2):
            return ap_2d.rearrange("p (i w) -> p i w", i=2) if i2 is None \
                else ap_2d.rearrange("p (i w) -> p i w", i=2)[:, :, i2]

        # ---- PSUM tags ----
        # sc0/sc1/sc2: rotating 2-bank score tiles (also reused by FFN
        # up/down accumulators after attention). aux: 1-bank tiles shared
        # by AV, projection fills and transposes (2 bufs).
        sc_ctr = [0]

        def sc_tile(width=2 * MM_N, dtype=F32):
            t = pall.tile([P, width], dtype, tag=f"sc{sc_ctr[0] % 3}",
                          name="ps_sc", padded_shape=[P, 2 * MM_N])
            sc_ctr[0] += 1
            return t

        def aux_tile(width=MM_N, dtype=F32):
            return pall.tile([P, width], dtype, tag="aux", bufs=2,
                             name="ps_aux", padded_shape=[P, MM_N])

        # ---- warmup: flip the HAM clock gate before real matmuls ----
        warm_sb = slot("g0", T_, BF16)
        nc.vector.memset(warm_sb[:, 0:MM_N], 0.0)
        for wi in range(12):
            wp = aux_tile()
            nc.tensor.matmul(wp, lhsT=warm_sb[:, 0:P], rhs=warm_sb[:, 0:MM_N],
                             start=True, stop=True)

        # ---- cold-start DMAs: xT8 + wq + wk split across 4 queues ----
        xT8_sb = [slot(f"xT8{c2}", 2 * T_, FP8) for c2 in range(KD2)]
        w_sb = {nm: [slot(f"w{nm}{c2}", 2 * D_, FP8) for c2 in range(KD2)]
                for nm in ("q", "k", "v")}
        # cold loads go ONLY on sync+gpsimd: a dma_start blocks its issuing
        # engine until ring space frees, and ScalarE must be free to start
        # the exp stream at ~14us.
        qs = [dma, dma2]
        qi = [0]

        def cold_load(dst, src):
            qs[qi[0] % 2].dma_start(out=dst, in_=src)
            qi[0] += 1

        def wslice(t_or_d, mlo, mhi):
            # column range [mlo*P, mhi*P) of both halves of a K-pair tile
            return t_or_d.rearrange("p (i w) -> p i w", i=2)[
                :, :, mlo * P:mhi * P]

        # order: everything Q0/K0 needs first (xT8 + m=0 slices of wq/wk),
        # then the rest by first-use time
        for c2 in range(KD2):
            cold_load(xT8_sb[c2], xT8_d[c2])
        for nm, dd in (("q", wq_d), ("k", wk_d)):
            for c2 in range(KD2):
                cold_load(wslice(w_sb[nm][c2], 0, 1), wslice(dd[c2], 0, 1))
        for nm, dd in (("q", wq_d), ("k", wk_d)):
            for c2 in range(KD2):
                cold_load(wslice(w_sb[nm][c2], 1, 8), wslice(dd[c2], 1, 8))
        for c2 in range(KD2):
            cold_load(w_sb["v"][c2], wv_d[c2])

        # ---- constants ----
        eps_t = consts.tile([P, 1], F32)
        nc.vector.memset(eps_t, 1e-5)
        attb_t = consts.tile([P, 1], F32)
        nc.vector.memset(attb_t, ATT_BIAS)
        zero_t = consts.tile([P, 1], F32)
        nc.vector.memset(zero_t, 0.0)
        ident = consts.tile([P, P], BF16)
        make_identity(nc, ident)
        if cfg["has_bq"]:
            bq_sb = consts.tile([P, KD], F32)
            dma.dma_start(out=bq_sb,
                          in_=bq_d.ap().rearrange("(m p) -> p m", p=P))
        if cfg["has_bk"]:
            bk_sb = consts.tile([P, KD], F32)
            dma.dma_start(out=bk_sb,
                          in_=bk_d.ap().rearrange("(m p) -> p m", p=P))
        if cfg["has_bup"]:
            bup_sb = consts.tile([P, KF], F32)
            dma3.dma_start(out=bup_sb,
                           in_=bup_d.ap().rearrange("(m p) -> p m", p=P))
        if cfg["has_mask"]:
            mask_sb = consts.tile([P, TB_ * SKC], F32)
            dma3.dma_start(out=mask_sb,
                           in_=mask_d.ap().rearrange("b (kc p) -> p (b kc)",
                                                     p=P))
            mask2_sb = consts.tile([P, TB_ * SKC], F32)
            nc.vector.tensor_scalar_add(out=mask2_sb, in0=mask_sb,
                                        scalar1=ATT_BIAS)

        def bcast_row(dram_vec, n):
            t = consts.tile([P, n], F32, name=f"bc_{dram_vec.name}")
            dma3.dma_start(out=t,
                           in_=dram_vec.ap().unsqueeze(0).to_broadcast([P, n]))
            return t

        bv_bc = bcast_row(bv_d, D_) if cfg["has_bv"] else None
        bo_bc = bcast_row(bo_d, D_) if cfg["has_bo"] else None
        bdn_bc = bcast_row(bdn_d, D_) if cfg["has_bdn"] else None
        g1_bc = bcast_row(g1_d, D_) if cfg["has_n1"] else None
        b1_bc = bcast_row(b1_d, D_) if cfg["has_n1"] else None
        g2_bc = bcast_row(g2_d, D_) if cfg["has_n2"] else None
        b2_bc = bcast_row(b2_d, D_) if cfg["has_n2"] else None

        qT_sb = [slot(f"qT{c}", T_, BF16) for c in range(KD)]
        kT_sb = [slot(f"kT{c}", T_, BF16) for c in range(KD)]
        Vp8_sb = [slot(f"Vp{c}", 2 * VROW, FP8) for c in range(TCH // 2)]

        HB = P // 2

        # ---- QKV projections (fp8 DoubleRow) ----
        def qk_proj(nm, m, t2):
            has_b = cfg["has_bq"] if nm == "q" else cfg["has_bk"]
            bias = (bq_sb if nm == "q" else bk_sb) if has_b else None
            pt = aux_tile()
            for c2 in range(KD2):
                nc.tensor.matmul(
                    pt[:, :NTW],
                    lhsT=pair(w_sb[nm][c2], slice(m * P, (m + 1) * P)),
                    rhs=pair(xT8_sb[c2], slice(t2 * MM_N, t2 * MM_N + NTW)),
                    start=(c2 == 0), stop=(c2 == KD2 - 1),
                    perf_mode=DR)
            sl = slice(t2 * MM_N, t2 * MM_N + NTW)
            dst = (qT_sb if nm == "q" else kT_sb)[m]
            if has_b:
                nc.vector.tensor_scalar_add(out=dst[:, sl], in0=pt[:, :NTW],
                                            scalar1=bias[:, m:m + 1])
            else:
                # vector only: an evict on ScalarE would head-of-line
                # block the exp stream behind this op's DMA-gated matmul
                nc.vector.tensor_copy(out=dst[:, sl], in_=pt[:, :NTW])

        # V token-major into V' ([v(64), 1] per head; 2^-7 scale on evict)
        def v_proj_tr(tr):
            vdst = Vp8_sb[tr // 2][:, (tr % 2) * VROW:(tr % 2 + 1) * VROW]
            vd3 = vdst.rearrange("p (h c) -> p h c", c=VW)
            for n2 in range(ND):
                pt = aux_tile()
                for c2 in range(KD2):
                    nc.tensor.matmul(
                        pt[:, :NDW],
                        lhsT=pair(xT8_sb[c2], slice(tr * P, (tr + 1) * P)),
                        rhs=pair(w_sb["v"][c2],
                                 slice(n2 * MM_N, n2 * MM_N + NDW)),
                        start=(c2 == 0), stop=(c2 == KD2 - 1),
                        perf_mode=DR)
                hpn = NDW // HD_   # heads per N tile
                src = pt[:, :NDW].rearrange("p (h c) -> p h c", c=HD_)
                if cfg["has_bv"]:
                    tmp = work.tile([P, NDW], F32, tag="vtmp", name="vtmp")
                    nc.vector.tensor_add(
                        out=tmp, in0=pt[:, :NDW],
                        in1=bv_bc[:, n2 * MM_N:n2 * MM_N + NDW])
                    src = tmp.rearrange("p (h c) -> p h c", c=HD_)
                nc.vector.tensor_scalar_mul(
                    out=vd3[:, n2 * hpn:(n2 + 1) * hpn, 0:HD_], in0=src,
                    scalar1=1.0 / WS)
            nc.vector.memset(vd3[:, :, HD_:VW], 1.0)  # ones cols

        # ---- attention machinery ----
        # wo loads follow wv on the bulk queues (needed mid-b1 for fills)
        wo_sb = []
        for c2 in range(KD2):
            t = slot(f"wo{c2}", 2 * D_, FP8)
            cold_load(t, wo_d[c2])
            wo_sb.append(t)
        oT8_sb = [slot(f"oT{c2}", 2 * T_, FP8) for c2 in range(KD2)]
        oTu_sb = [slot(f"oTu{hc}", T_, BF16) for hc in range(KD)]
        den_d = dramp.tile([H_, T_], F32, tag="den_d", name="den_d")
        rb_sb = {}

        def at_tile():
            return attnp.tile([P, 2 * S_], FP8, tag="attnT", bufs=10 + 2,
                              name="attnT")

        def attn_scores(b, hc):
            """Row-packed scores for head pair hc: 4 psum tiles
            (E-kc01, O-kc01, E-kc23, O-kc23), one exp each -> 4 at tiles
            (kc-paired fp8, ready for DR AV)."""
            ats = []
            for half in range(2):          # kc01 / kc23
                pts = [sc_tile(), sc_tile()]   # [even-head, odd-head]
                for k2 in range(2):
                    kc = 2 * half + k2
                    for par in range(HPC):
                        # K=64 at base partition 0/64: the row group
                        # auto-derives, so the two heads' matmuls share
                        # the array concurrently
                        nc.tensor.matmul(
                            pts[par][:, k2 * S_:(k2 + 1) * S_],
                            lhsT=kT_sb[hc][par * HD_:(par + 1) * HD_,
                                           b * S_ + kc * P:
                                           b * S_ + (kc + 1) * P],
                            rhs=qT_sb[hc][par * HD_:(par + 1) * HD_,
                                          b * S_:(b + 1) * S_],
                            start=True, stop=True)
                for par in range(HPC):
                    at = at_tile()
                    if cfg["has_mask"]:
                        for k2 in range(2):
                            kc = 2 * half + k2
                            nc.scalar.activation(
                                out=at[:, k2 * S_:(k2 + 1) * S_],
                                in_=pts[par][:, k2 * S_:(k2 + 1) * S_],
                                func=mybir.ActivationFunctionType.Exp,
                                bias=mask2_sb[:, b * SKC + kc:
                                              b * SKC + kc + 1],
                                scale=EXP_SCALE)
                    else:
                        nc.scalar.activation(
                            out=at, in_=pts[par][:, 0:2 * S_],
                            func=mybir.ActivationFunctionType.Exp,
                            bias=attb_t, scale=EXP_SCALE)
                    ats.append(at)
            # ats = [E-kc01, O-kc01, E-kc23, O-kc23]
            return ats

        def attn_av(b, hc, ats):
            for par in range(HPC):
                h = hc * HPC + par
                pv = aux_tile()
                for half in range(2):
                    nc.tensor.matmul(
                        pv[0:VW, :S_],
                        lhsT=pair(Vp8_sb[b * SKC2 + half],
                                  slice(h * VW, (h + 1) * VW)),
                        rhs=pair(ats[2 * half + par], None),
                        start=(half == 0), stop=(half == 1),
                        perf_mode=DR)
                ho = par * HD_
                nc.vector.tensor_copy(
                    out=oTu_sb[hc][ho:ho + HD_, b * S_:(b + 1) * S_],
                    in_=pv[0:HD_, :S_])
                rs = attn2.tile([1, S_], F32, tag="rs", bufs=3, name="rs")
                nc.vector.tensor_copy(out=rs, in_=pv[HD_:VW, :S_])
                dma2.dma_start(out=den_d[h:h + 1, b * S_:(b + 1) * S_],
                               in_=rs)

        def rb_load(b, hc):
            # broadcast this chunk's denominators back from DRAM
            sl = slice(b * S_, (b + 1) * S_)
            rb = rbp.tile([P, S_], F32, tag=f"rb{hc % 4}", name="rb")
            rb_sb[hc] = rb
            for h2 in range(HPC):
                dma2.dma_start(
                    out=rb[h2 * HD_:(h2 + 1) * HD_, :],
                    in_=den_d[HPC * hc + h2:HPC * hc + h2 + 1, sl]
                    .to_broadcast([HD_, S_]))

        def recip_mul(b, hc):
            # reciprocal + normalize one feature chunk: oT8 = oTu / den.
            # Runs one chunk behind rb_load so the DMA latency is hidden.
            sl = slice(b * S_, (b + 1) * S_)
            rb = rb_sb[hc]
            nc.vector.reciprocal_approx_fast(out=rb, in_=rb)
            nc.vector.tensor_mul(
                out=oT8_sb[hc // 2][:, (hc % 2) * T_ + b * S_:
                                    (hc % 2) * T_ + (b + 1) * S_],
                in0=oTu_sb[hc][:, sl], in1=rb)

        # O-proj machinery; LN1 computes rstd with a vector-side Newton
        # rsqrt (seed 2^-7: the LN1 input is 2^7-scaled, so var ~= 2^14)
        # so no ScalarE act-table switch ever interrupts the exp stream.
        xm_bf = {}
        xmT8_sb = [slot(f"xmT{c2}", 2 * T_, FP8) for c2 in range(KD2)]

        def newton_rstd(v_col, eng):
            # 1/sqrt(v) for v ~ 2^14 * [0.8, 2.0]; 3 iterations to fp32-ish
            y = statp.tile([P, 1], F32, tag="nwy", name="nwy")
            t = statp.tile([P, 1], F32, tag="nwt", name="nwt")
            eng.memset(y, 2.0 ** -7)
            for _ in range(3):
                eng.tensor_mul(out=t, in0=y, in1=y)
                eng.tensor_mul(out=t, in0=t, in1=v_col)
                eng.tensor_scalar(
                    out=t, in0=t, scalar1=-0.5, scalar2=1.5,
                    op0=mybir.AluOpType.mult, op1=mybir.AluOpType.add)
                eng.tensor_mul(out=y, in0=y, in1=t)
            return y

        def ln1_tr(tr):
            # in-place LayerNorm on the bf16 x_medium tile (vector-only;
            # gpsimd bulk elementwise is ~17x slower than DVE)
            xm = xm_bf[tr]
            bw = min(512, D_)
            nsub = _ceil_div(D_, bw)
            st = statp.tile([P, nsub, 6], F32, tag="bnst", name="bnst")
            for i in range(nsub):
                nc.vector.bn_stats(out=st[:, i, :],
                                   in_=xm[:, i * bw:(i + 1) * bw])
            mv = statp.tile([P, 2], F32, tag="bnmv", name="bnmv")
            nc.vector.bn_aggr(out=mv, in_=st)
            rstd = newton_rstd(mv[:, 1:2], nc.vector)
            if cfg["has_n1"]:
                tmp = statp.tile([P, D_], F32, tag="lntmp", name="lntmp")
                nc.vector.tensor_scalar(
                    out=tmp, in0=xm, scalar1=mv[:, 0:1], scalar2=rstd,
                    op0=mybir.AluOpType.subtract, op1=mybir.AluOpType.mult)
                nc.vector.tensor_mul(out=tmp, in0=tmp, in1=g1_bc)
                nc.vector.tensor_add(out=xm, in0=tmp, in1=b1_bc)
            else:
                nc.vector.tensor_scalar(
                    out=xm, in0=xm, scalar1=mv[:, 0:1], scalar2=rstd,
                    op0=mybir.AluOpType.subtract, op1=mybir.AluOpType.mult)

        xr_tiles = {}

        def xr_load(tr):
            xt = xrp.tile([P, D_], BF16, tag="xrt", name="xrt")
            dma2.dma_start(out=xt, in_=xr_d[tr * P:(tr + 1) * P, :])
            xr_tiles[tr] = xt

        def o_mm_tr(tr):
            # O-projection matmuls + residual add -> bf16 xm (pre-LN)
            xt = xr_tiles[tr]
            xm = slot(f"qT{tr}", D_, BF16)   # reuse qT slot (scores done)
            xm_bf[tr] = xm
            for n2 in range(ND):
                pt = aux_tile()
                for c2 in range(KD2):
                    nc.tensor.matmul(
                        pt[:, :NDW],
                        lhsT=pair(oT8_sb[c2], slice(tr * P, (tr + 1) * P)),
                        rhs=pair(wo_sb[c2],
                                 slice(n2 * MM_N, n2 * MM_N + NDW)),
                        start=(c2 == 0), stop=(c2 == KD2 - 1),
                        perf_mode=DR)
                nc.vector.tensor_add(out=xm[:, n2 * MM_N:n2 * MM_N + NDW],
                                     in0=pt[:, :NDW],
                                     in1=xt[:, n2 * MM_N:n2 * MM_N + NDW])
                if cfg["has_bo"]:
                    nc.vector.tensor_add(
                        out=xm[:, n2 * MM_N:n2 * MM_N + NDW],
                        in0=xm[:, n2 * MM_N:n2 * MM_N + NDW],
                        in1=bo_bc[:, n2 * MM_N:n2 * MM_N + NDW])

        def o_proj_tr(tr):
            o_mm_tr(tr)
            ln1_tr(tr)

        def transpose_tr(tr):
            # PE transposes, evicted 2-at-a-time with a 3D [128,2,128] AP
            # (DVE only: gpsimd has no PSUM port)
            for c2 in range(KD2):
                pt = pall.tile([P, 2 * P], BF16, tag="aux", bufs=2,
                               name="ps_t", padded_shape=[P, MM_N])
                for j in range(2):
                    c = 2 * c2 + j
                    nc.tensor.transpose(pt[:, j * P:(j + 1) * P],
                                        xm_bf[tr][:, c * P:(c + 1) * P],
                                        ident)
                dst = xmT8_sb[c2].rearrange(
                    "p (i w) -> p i w", i=2)[:, :, tr * P:(tr + 1) * P]
                nc.vector.tensor_copy(out=dst,
                                      in_=pt.rearrange("p (i w) -> p i w",
                                                       i=2))

        # ---- FFN up helpers (t2-split halves) ----
        wup_sb = {}

        def wup_load(i, tag, cold=False):
            t = slot(tag, 2 * UPW, FP8)
            if cold:
                cold_load(t, wup_d[i])
            else:
                dma.dma_start(out=t, in_=wup_d[i])
            wup_sb[i] = t

        # the fm<16 half of wup goes into the idle dn tags NOW (trickles
        # in during b0 attention) so FFN up can start the moment the
        # attention loop ends
        for c2 in range(KD2):
            wup_load(c2 * UPT + 0, f"dn{c2}", cold=True)
        for c2 in range(KD2):
            wup_load(c2 * UPT + 1, f"dn{4 + c2}", cold=True)

        def wup_lhsT(c2, fm):
            i = c2 * UPT + (fm * P) // UPW
            o = (fm * P) % UPW
            return pair(wup_sb[i], slice(o, o + P))

        gT_sb = {}
        # tag order matters: oT tags free only after o_proj_tr(7), which is
        # woven at up-t2=0 fm==23 -> oT tags must serve fm>=28 only
        g_tags = ([f"g{c}" for c in range(KF - KD - 2 * KD2)]
                  + [f"wv{c2}" for c2 in range(KD2)]
                  + [f"oTu{hc}" for hc in range(KD)]
                  + [f"oT{c2}" for c2 in range(KD2)])

        def up_half(fm, t2):
            pt = aux_tile()
            for c2 in range(KD2):
                nc.tensor.matmul(
                    pt[:, :NTW],
                    lhsT=wup_lhsT(c2, fm),
                    rhs=pair(xmT8_sb[c2],
                             slice(t2 * MM_N, t2 * MM_N + NTW)),
                    start=(c2 == 0), stop=(c2 == KD2 - 1),
                    perf_mode=DR)
            if fm not in gT_sb:
                gT_sb[fm] = slot(g_tags[fm], T_, BF16)
            nc.scalar.activation(
                out=gT_sb[fm][:, t2 * MM_N:t2 * MM_N + NTW],
                in_=pt[:, :NTW],
                func=mybir.ActivationFunctionType.Gelu,
                bias=(bup_sb[:, fm:fm + 1] if cfg["has_bup"] else zero_t),
                scale=1.0 / WS)

        # ---- emit: Q0/K0 then the exp-stream-driven attention loop ----
        qk_proj("q", 0, 0)
        qk_proj("k", 0, 0)

        # fills per (b, pair-index): list of thunks. Constraints:
        #  - v(0..3) emitted by slot (0,1) (first AV dequeues at (0,2));
        #    v(4..7) by slot (1,1).
        #  - qk(m,0) by slot (0,m-1); qk(m,1) by slot (1,m-1) (or in b0).
        def F_qk(m, t2):
            return lambda: (qk_proj("q", m, t2), qk_proj("k", m, t2))

        def F_v(tr):
            return lambda: v_proj_tr(tr)

        fills = {
            (0, 0): [F_qk(1, 0), F_v(0), F_v(1)],
            (0, 1): [F_qk(2, 0), F_v(2), F_v(3)],
            (0, 2): [F_qk(3, 0)], (0, 3): [F_qk(4, 0)],
            (0, 4): [F_qk(5, 0)], (0, 5): [F_qk(6, 0)],
            (0, 6): [F_qk(7, 0), F_qk(0, 1)],
            (0, 7): [F_qk(1, 1), F_v(4)],
            (1, 0): [F_v(5), F_qk(2, 1)],
            (1, 1): [F_v(6), F_v(7), F_qk(3, 1)],
            (1, 2): [F_qk(4, 1)],
            (1, 3): [F_qk(5, 1)],
            (1, 4): [F_qk(6, 1)],
            (1, 5): [F_qk(7, 1)],
        }

        avq = []   # 2-deep AV lag so V fills land before the first AV
        rmq = []   # recip_mul runs one chunk behind rb_load

        def av_dequeue():
            pb, phc, ats = avq.pop(0)
            attn_av(pb, phc, ats)
            rb_load(pb, phc)
            rmq.append((pb, phc))
            if len(rmq) >= 2:
                recip_mul(*rmq.pop(0))

        for b in range(TB_):
            for hc in range(KD):
                ats = attn_scores(b, hc)
                avq.append((b, hc, ats))
                if len(avq) > 2:
                    av_dequeue()
                for f in fills.get((b, hc), []):
                    f()
        while avq:
            av_dequeue()
        while rmq:
            recip_mul(*rmq.pop(0))

        # ---- post-attention ----
        # wup fm>=16 half: j=2 blocks into wq tags (free after the
        # deferred QK fills), j=3 into wk tags.
        for tr in range(TCH):
            xr_load(tr)
        for c2 in range(KD2):
            wup_load(c2 * UPT + 2, f"wq{c2}")
        for c2 in range(KD2):
            wup_load(c2 * UPT + 3, f"wk{c2}")
        # O-proj/LN1/transpose of trs 0-3 unlock up-t2=0 (its xmT8 column
        # slice only spans batch-0 tokens)
        o_proj_tr(0)
        o_proj_tr(1)
        transpose_tr(0)
        o_proj_tr(2)
        transpose_tr(1)
        o_proj_tr(3)
        transpose_tr(2)
        transpose_tr(3)

        # FFN down weight loads issued early so the DMA hides under up
        # wo tags free only after o_proj_tr(7): keep them last so the sync
        # queue isn't blocked mid-stream waiting on the weave
        dn_tags = ([f"Vp{c}" for c in range(TCH // 2)]
                   + [f"xT8{c2}" for c2 in range(KD2)]
                   + [f"kT{c}" for c in range(KD)]
                   + [f"dn{i}" for i in range(8)]
                   + [f"wo{c2}" for c2 in range(KD2)])
        wdn_sb = []
        for fc in range(KF):
            if fc < len(dn_tags):
                t = slot(dn_tags[fc], D_, BF16)
            else:
                t = rbp.tile([P, D_], BF16, tag=f"rb{fc - len(dn_tags)}",
                             name="t_wdn_rb")
            # sync queue only: scalar must stay free for the gelu stream
            dma.dma_start(out=t, in_=wdn_d[fc * P:(fc + 1) * P, :])
            wdn_sb.append(t)

        # up-t2=0 matmuls hide the vector-bound O/LN1 chains of trs 4-7
        weave = {2: lambda: o_proj_tr(4), 7: lambda: transpose_tr(4),
                 9: lambda: o_proj_tr(5), 14: lambda: transpose_tr(5),
                 16: lambda: o_proj_tr(6), 21: lambda: transpose_tr(6),
                 23: lambda: o_proj_tr(7), 28: lambda: transpose_tr(7)}
        for fm in range(KF):
            up_half(fm, 0)
            if fm in weave:
                weave[fm]()
        for fm in range(KF):
            up_half(fm, 1)

        # ---- FFN down (bf16) + residual + LN2 -> out, incremental ----
        def layer_norm_apply(src_t, dst, mv, rstd, g_bc, b_bc):
            if g_bc is None:
                nc.vector.tensor_scalar(
                    out=dst, in0=src_t, scalar1=mv[:, 0:1], scalar2=rstd,
                    op0=mybir.AluOpType.subtract, op1=mybir.AluOpType.mult)
            else:
                tmp = statp.tile([P, D_], F32, tag="lntmp", name="lntmp")
                nc.vector.tensor_scalar(
                    out=tmp, in0=src_t, scalar1=mv[:, 0:1], scalar2=rstd,
                    op0=mybir.AluOpType.subtract, op1=mybir.AluOpType.mult)
                nc.vector.tensor_mul(out=tmp, in0=tmp, in1=g_bc)
                nc.vector.tensor_add(out=dst, in0=tmp, in1=b_bc)

        for tr in range(TCH):
            dsb = work.tile([P, D_], F32, tag="acc", name="dsb")
            st = statp.tile([P, ND, 6], F32, tag="bnst", name="bnst")
            pt = sc_tile()
            for n2 in range(ND):
                for fc in range(KF):
                    nc.tensor.matmul(
                        pt[:, n2 * MM_N:n2 * MM_N + NDW],
                        lhsT=gT_sb[fc][:, tr * P:(tr + 1) * P],
                        rhs=wdn_sb[fc][:, n2 * MM_N:n2 * MM_N + NDW],
                        start=(fc == 0), stop=(fc == KF - 1))
                # evict+add+stats per half so only the last half's chain
                # is exposed after the final matmul
                sl = slice(n2 * MM_N, n2 * MM_N + NDW)
                nc.vector.tensor_add(out=dsb[:, sl], in0=pt[:, sl],
                                     in1=xm_bf[tr][:, sl])
                if cfg["has_bdn"]:
                    nc.vector.tensor_add(out=dsb[:, sl], in0=dsb[:, sl],
                                         in1=bdn_bc[:, sl])
                nc.vector.bn_stats(out=st[:, n2, :], in_=dsb[:, sl])
            mv = statp.tile([P, 2], F32, tag="bnmv", name="bnmv")
            nc.vector.bn_aggr(out=mv, in_=st)
            rstd = statp.tile([P, 1], F32, tag="rstd", name="rstd")
            nc.scalar.activation(out=rstd, in_=mv[:, 1:2],
                                 func=mybir.ActivationFunctionType.Sqrt,
                                 bias=eps_t, scale=1.0)
            nc.vector.reciprocal(out=rstd, in_=rstd)
            ot = outp.tile([P, D_], F32, tag="ot", name="ot")
            layer_norm_apply(dsb, ot, mv, rstd,
                             g2_bc if cfg["has_n2"] else None,
                             b2_bc if cfg["has_n2"] else None)
            if tr < TCH - 1:
                dma.dma_start(out=out_d[tr * P:(tr + 1) * P, :], in_=ot)
            else:
                # last chunk is latency-exposed: split across HW queues
                qw = D_ // 4
                engs = (nc.sync, nc.scalar, nc.sync, nc.scalar)
                for qi, eng in enumerate(engs):
                    eng.dma_start(
                        out=out_d[tr * P:(tr + 1) * P,
                                  qi * qw:(qi + 1) * qw],
                        in_=ot[:, qi * qw:(qi + 1) * qw])

    nc.finalize()
    return nc


_PROGRAM_CACHE = {}


def _get_program(cfg_key, cfg):
    if cfg_key not in _PROGRAM_CACHE:
        _PROGRAM_CACHE[cfg_key] = build_program(cfg)
    return _PROGRAM_CACHE[cfg_key]


def _swz(w, npairs, width):
    """[rows, cols] -> [npairs, 128, 2*cols] K-paired contiguous."""
    return np.ascontiguousarray(
        w.reshape(npairs, 2, P, width).transpose(0, 2, 1, 3)
        .reshape(npairs, P, 2 * width))


def make_in_maps(inputs):
    f32 = np.float32
    x = np.asarray(inputs["x"], f32)
    scale = 1.0 / np.sqrt(float(inputs["head_dim"]))

    def merged(w, a, b):
        return (np.asarray(w, f32)
                + np.asarray(a, f32) @ np.asarray(b, f32))

    KD2 = D // P // 2
    wq = _swz((merged(inputs["w_q"], inputs["w_q_lora_a"],
                      inputs["w_q_lora_b"]) * (scale * QS)).astype(NP_FP8),
              KD2, D)
    wk = _swz((merged(inputs["w_k"], inputs["w_k_lora_a"],
                      inputs["w_k_lora_b"]) * WS).astype(NP_FP8), KD2, D)
    wv = _swz((merged(inputs["w_v"], inputs["w_v_lora_a"],
                      inputs["w_v_lora_b"]) * WS).astype(NP_FP8), KD2, D)
    wo = _swz((merged(inputs["w_o"], inputs["w_o_lora_a"],
                      inputs["w_o_lora_b"]) * WS).astype(NP_FP8), KD2, D)
    wup8 = (merged(inputs["w_up"], inputs["w_up_lora_a"],
                   inputs["w_up_lora_b"]) * WS).astype(NP_FP8)
    UPW = 1024
    UPT = F // UPW
    wup = np.ascontiguousarray(
        wup8.reshape(KD2, 2, P, UPT, UPW).transpose(0, 3, 2, 1, 4)
        .reshape(KD2 * UPT, P, 2 * UPW))
    wdn = merged(inputs["w_down"], inputs["w_down_lora_a"],
                 inputs["w_down_lora_b"]).astype(NP_BF16)
    mask = np.asarray(inputs["attention_mask"], f32)

    common = {
        "wq": wq, "wk": wk, "wv": wv, "wo": wo, "wup": wup, "wdn": wdn,
        "bq": (np.asarray(inputs["b_q"], f32) * (scale * QS)).astype(f32),
        "bk": (np.asarray(inputs["b_k"], f32) * WS).astype(f32),
        "bup": np.asarray(inputs["b_up"], f32),
        "bv": np.asarray(inputs["b_v"], f32),
        "bo": np.asarray(inputs["b_o"], f32),
        "bdn": np.asarray(inputs["b_down"], f32),
        "g1": np.asarray(inputs["norm_weight_1"], f32),
        "b1": np.asarray(inputs["norm_bias_1"], f32),
        "g2": np.asarray(inputs["norm_weight_2"], f32),
        "b2": np.asarray(inputs["norm_bias_2"], f32),
    }
    in_maps = []
    for i in range(N_CORES):
        xc = x[i * TB:(i + 1) * TB].reshape(T, D)
        m = dict(common)
        m["xT8"] = _swz(np.ascontiguousarray(xc.T).astype(NP_FP8), KD2, T)
        m["xr"] = (np.ascontiguousarray(xc) * WS).astype(NP_BF16)
        m["maskT"] = np.ascontiguousarray(mask[i * TB:(i + 1) * TB, 0, 0, :])
        in_maps.append(m)
    return in_maps


def full_cfg(inputs):
    f32 = np.float32
    return {
        "D": D, "F": F, "T": T, "TB": TB, "H": H, "HD": HD,
        "has_bq": bool(np.any(np.asarray(inputs["b_q"], f32))),
        "has_bk": bool(np.any(np.asarray(inputs["b_k"], f32))),
        "has_bup": bool(np.any(np.asarray(inputs["b_up"], f32))),
        "has_mask": bool(np.any(np.asarray(inputs["attention_mask"], f32))),
        "has_bv": bool(np.any(np.asarray(inputs["b_v"], f32))),
        "has_bo": bool(np.any(np.asarray(inputs["b_o"], f32))),
        "has_bdn": bool(np.any(np.asarray(inputs["b_down"], f32))),
        "has_n1": bool(np.any(np.asarray(inputs["norm_weight_1"], f32) != 1.0)
                       or np.any(np.asarray(inputs["norm_bias_1"], f32))),
        "has_n2": bool(np.any(np.asarray(inputs["norm_weight_2"], f32) != 1.0)
                       or np.any(np.asarray(inputs["norm_bias_2"], f32))),
    }


def run_on_hw(inputs, trace=False, tmpdir=None):
    cfg = full_cfg(inputs)
    cfg_key = tuple(sorted((k, v) for k, v in cfg.items()
                           if not isinstance(v, set)))
    nc = _get_program(cfg_key, cfg)
    in_maps = make_in_maps(inputs)
    kw = {}
    if trace:
        kw = {"trace": True, "tmpdir": tmpdir}
    res = run_bass_kernel_spmd(nc, in_maps, core_ids=list(range(N_CORES)),
                               **kw)
    out = np.empty((B, S, D), np.float32)
    for i in range(N_CORES):
        out[i * TB:(i + 1) * TB] = res.results[i]["out"].reshape(TB, S, D)
    return out, res


def kernel(**inputs):
    out, _ = run_on_hw(inputs)
    return out


# revision 46
# speedup vs baseline: 1.0687x; 1.0415x over previous
"""Fused RoBERTa layer (attention + FFN, LoRA merged) on 8 Trainium2 cores.

Sharding: pure data-parallel over batch (16 batches -> 2 per core), no
collectives. LoRA merged into base weights on host; 1/sqrt(hd) folded into
w_q.

v2 layout (vs v1): attention is organized around the ScalarE exp stream
(the hard floor: 64 exps of [128,1024] ~= 71us). Everything else hides
under it:
  - Scores are 2-head row-packed: kT is a single [128, T] tile per head
    pair (even head on partitions 0-63, odd on 64-127); each score matmul
    contracts K=64 via tile_position (0,0)/(64,0) so the two heads' score
    matmuls run CONCURRENTLY in the PE array (no zero-padding waste).
  - Score PSUM tiles rotate through 3 tags (2 banks each) so the next
    tile's matmuls never wait on the current exp (WAR double-buffer+1).
  - ScalarE does exp ONLY during attention. Denominator row copies go to
    vector/gpsimd; transpose evicts are batched [128,2,128] vector ops.
  - Attention starts at ~8us: only Q0/K0 are emitted before it. The rest
    of Q/K (t2=0 then deferred t2=1), all of V, O-proj of batch 0, and
    the LN1 transposes are PE filler slotted between score/AV matmuls.
  - 12 warmup matmuls on garbage data at t=0 flip the HAM clock gate to
    8/8 before real matmuls arrive; initial DMAs split across 4 queues.

fp8 strategy (DoubleRow double-pumping) as v1: QKV / AV / O-proj / FFN-up
run fp8e4m3 DR (weights pre-scaled 2^7, 2^10 for w_q; inverse scales
folded into exp/gelu/evict scales). FFN-down stays bf16 (fp8 breaks the
2e-2 gate; verified by exact numpy emulation of TRN DR semantics).
Attention normalization: V' carries a ones column so AV emits
unnormalized o rows + a denominator row; dens round-trip through DRAM
([H,T] tile) and come back partition-broadcast, one chunk behind, so the
DMA latency is hidden.
"""

import math
import sys

sys.path.insert(0, "/opt/trn_rl_repo")

import numpy as np
import ml_dtypes

import concourse.bacc as bacc
import concourse.bass as bass
import concourse.tile as tile
from concourse import mybir
from concourse.bass_utils import run_bass_kernel_spmd
from concourse.masks import make_identity

BF16 = mybir.dt.bfloat16
FP8 = mybir.dt.float8e4
F32 = mybir.dt.float32
NP_BF16 = np.dtype(ml_dtypes.bfloat16)
NP_FP8 = np.dtype(ml_dtypes.float8_e4m3)

B, S, D, H, HD, F = 16, 512, 1024, 16, 64, 4096
N_CORES = 8
TB = B // N_CORES
T = TB * S

MM_N = 512
P = 128

WSHIFT = 7
WS = float(2.0 ** WSHIFT)
QSHIFT = 10
QS = float(2.0 ** QSHIFT)
EXP_SCALE = float(2.0 ** (-(WSHIFT + QSHIFT)))
ATT_BIAS = -9 * math.log(2.0)


def _ceil_div(a, b):
    return (a + b - 1) // b


def build_program(cfg):
    D_, F_, T_, TB_, H_, HD_ = (cfg["D"], cfg["F"], cfg["T"], cfg["TB"],
                                cfg["H"], cfg["HD"])
    S_ = T_ // TB_
    KD = D_ // P
    KD2 = KD // 2
    KF = F_ // P
    TCH = T_ // P
    NT = _ceil_div(T_, MM_N)
    NTW = min(MM_N, T_)
    ND = _ceil_div(D_, MM_N)
    NDW = min(MM_N, D_)
    SKC = S_ // P
    SKC2 = SKC // 2
    HPC = P // HD_             # heads per 128-partition chunk (=2)
    VW = HD_ + 1               # V' per-head width (ones column)
    VROW = H_ * VW             # V' row width for one key chunk
    UPW = 1024
    UPT = F_ // UPW

    nc = bacc.Bacc("TRN2", target_bir_lowering=False, debug=False,
                   num_devices=N_CORES)

    # ---- DRAM I/O (fp8 tensors pre-swizzled on host: [ntile, 128, W]) ----
    xT8_d = nc.dram_tensor("xT8", [KD2, P, 2 * T_], FP8,
                           kind="ExternalInput")
    xr_d = nc.dram_tensor("xr", [T_, D_], BF16, kind="ExternalInput")
    wq_d = nc.dram_tensor("wq", [KD2, P, 2 * D_], FP8, kind="ExternalInput")
    wk_d = nc.dram_tensor("wk", [KD2, P, 2 * D_], FP8, kind="ExternalInput")
    wv_d = nc.dram_tensor("wv", [KD2, P, 2 * D_], FP8, kind="ExternalInput")
    wo_d = nc.dram_tensor("wo", [KD2, P, 2 * D_], FP8, kind="ExternalInput")
    wup_d = nc.dram_tensor("wup", [KD2 * UPT, P, 2 * UPW], FP8,
                           kind="ExternalInput")
    wdn_d = nc.dram_tensor("wdn", [F_, D_], BF16, kind="ExternalInput")
    bq_d = nc.dram_tensor("bq", [D_], F32, kind="ExternalInput")
    bk_d = nc.dram_tensor("bk", [D_], F32, kind="ExternalInput")
    bup_d = nc.dram_tensor("bup", [F_], F32, kind="ExternalInput")
    mask_d = nc.dram_tensor("maskT", [TB_, S_], F32, kind="ExternalInput")
    bv_d = nc.dram_tensor("bv", [D_], F32, kind="ExternalInput")
    bo_d = nc.dram_tensor("bo", [D_], F32, kind="ExternalInput")
    bdn_d = nc.dram_tensor("bdn", [D_], F32, kind="ExternalInput")
    g1_d = nc.dram_tensor("g1", [D_], F32, kind="ExternalInput")
    b1_d = nc.dram_tensor("b1", [D_], F32, kind="ExternalInput")
    g2_d = nc.dram_tensor("g2", [D_], F32, kind="ExternalInput")
    b2_d = nc.dram_tensor("b2", [D_], F32, kind="ExternalInput")
    out_d = nc.dram_tensor("out", [T_, D_], F32, kind="ExternalOutput")

    DR = mybir.MatmulPerfMode.DoubleRow

    with tile.TileContext(nc) as tc, \
         tc.tile_pool(name="consts", bufs=1) as consts, \
         tc.tile_pool(name="slab", bufs=1) as slab, \
         tc.tile_pool(name="pall", bufs=1, space="PSUM") as pall, \
         tc.tile_pool(name="work", bufs=2) as work, \
         tc.tile_pool(name="xrp", bufs=2) as xrp, \
         tc.tile_pool(name="attnp", bufs=1) as attnp, \
         tc.tile_pool(name="attn2", bufs=4) as attn2, \
         tc.tile_pool(name="rbp", bufs=1) as rbp, \
         tc.tile_pool(name="statp", bufs=4) as statp, \
         tc.tile_pool(name="outp", bufs=2) as outp, \
         tc.tile_pool(name="dramp", bufs=2, space="DRAM") as dramp:

        dma = nc.sync          # bulk loads
        dma2 = nc.gpsimd       # latency-bound small DMAs
        dma3 = nc.scalar       # second bulk queue (cold start)

        def slot(tag, width, dtype):
            return slab.tile([P, width], dtype, tag=tag, name=f"t_{tag}")

        def pair(ap_2d, i2):
            return ap_2d.rearrange("p (i w) -> p i w", i=2) if i2 is None \
                else ap_2d.rearrange("p (i w) -> p i w", i=2)[:, :, i2]

        # ---- PSUM tags ----
        # sc0/sc1/sc2: rotating 2-bank score tiles (also reused by FFN
        # up/down accumulators after attention). aux: 1-bank tiles shared
        # by AV, projection fills and transposes (2 bufs).
        sc_ctr = [0]

        def sc_tile(width=2 * MM_N, dtype=F32):
            t = pall.tile([P, width], dtype, tag=f"sc{sc_ctr[0] % 3}",
                          name="ps_sc", padded_shape=[P, 2 * MM_N])
            sc_ctr[0] += 1
            return t

        def aux_tile(width=MM_N, dtype=F32):
            return pall.tile([P, width], dtype, tag="aux", bufs=2,
                             name="ps_aux", padded_shape=[P, MM_N])

        # ---- warmup: flip the HAM clock gate before real matmuls ----
        warm_sb = slot("g0", T_, BF16)
        nc.vector.memset(warm_sb[:, 0:MM_N], 0.0)
        for wi in range(12):
            wp = aux_tile()
            nc.tensor.matmul(wp, lhsT=warm_sb[:, 0:P], rhs=warm_sb[:, 0:MM_N],
                             start=True, stop=True)

        # ---- cold-start DMAs: xT8 + wq + wk split across 4 queues ----
        xT8_sb = [slot(f"xT8{c2}", 2 * T_, FP8) for c2 in range(KD2)]
        w_sb = {nm: [slot(f"w{nm}{c2}", 2 * D_, FP8) for c2 in range(KD2)]
                for nm in ("q", "k", "v")}
        # cold loads go ONLY on sync+gpsimd: a dma_start blocks its issuing
        # engine until ring space frees, and ScalarE must be free to start
        # the exp stream at ~14us.
        qs = [dma, dma2]
        qi = [0]

        def cold_load(dst, src):
            qs[qi[0] % 2].dma_start(out=dst, in_=src)
            qi[0] += 1

        def wslice(t_or_d, mlo, mhi):
            # column range [mlo*P, mhi*P) of both halves of a K-pair tile
            return t_or_d.rearrange("p (i w) -> p i w", i=2)[
                :, :, mlo * P:mhi * P]

        # order: everything Q0/K0 needs first (xT8 + m=0 slices of wq/wk),
        # then the rest by first-use time
        for c2 in range(KD2):
            cold_load(xT8_sb[c2], xT8_d[c2])
        for nm, dd in (("q", wq_d), ("k", wk_d)):
            for c2 in range(KD2):
                cold_load(wslice(w_sb[nm][c2], 0, 1), wslice(dd[c2], 0, 1))
        for nm, dd in (("q", wq_d), ("k", wk_d)):
            for c2 in range(KD2):
                cold_load(wslice(w_sb[nm][c2], 1, 8), wslice(dd[c2], 1, 8))
        for c2 in range(KD2):
            cold_load(w_sb["v"][c2], wv_d[c2])

        # ---- constants ----
        eps_t = consts.tile([P, 1], F32)
        nc.vector.memset(eps_t, 1e-5)
        attb_t = consts.tile([P, 1], F32)
        nc.vector.memset(attb_t, ATT_BIAS)
        zero_t = consts.tile([P, 1], F32)
        nc.vector.memset(zero_t, 0.0)
        ident = consts.tile([P, P], BF16)
        make_identity(nc, ident)
        if cfg["has_bq"]:
            bq_sb = consts.tile([P, KD], F32)
            dma.dma_start(out=bq_sb,
                          in_=bq_d.ap().rearrange("(m p) -> p m", p=P))
        if cfg["has_bk"]:
            bk_sb = consts.tile([P, KD], F32)
            dma.dma_start(out=bk_sb,
                          in_=bk_d.ap().rearrange("(m p) -> p m", p=P))
        if cfg["has_bup"]:
            bup_sb = consts.tile([P, KF], F32)
            dma3.dma_start(out=bup_sb,
                           in_=bup_d.ap().rearrange("(m p) -> p m", p=P))
        if cfg["has_mask"]:
            mask_sb = consts.tile([P, TB_ * SKC], F32)
            dma3.dma_start(out=mask_sb,
                           in_=mask_d.ap().rearrange("b (kc p) -> p (b kc)",
                                                     p=P))
            mask2_sb = consts.tile([P, TB_ * SKC], F32)
            nc.vector.tensor_scalar_add(out=mask2_sb, in0=mask_sb,
                                        scalar1=ATT_BIAS)

        def bcast_row(dram_vec, n):
            t = consts.tile([P, n], F32, name=f"bc_{dram_vec.name}")
            dma3.dma_start(out=t,
                           in_=dram_vec.ap().unsqueeze(0).to_broadcast([P, n]))
            return t

        bv_bc = bcast_row(bv_d, D_) if cfg["has_bv"] else None
        bo_bc = bcast_row(bo_d, D_) if cfg["has_bo"] else None
        bdn_bc = bcast_row(bdn_d, D_) if cfg["has_bdn"] else None
        g1_bc = bcast_row(g1_d, D_) if cfg["has_n1"] else None
        b1_bc = bcast_row(b1_d, D_) if cfg["has_n1"] else None
        g2_bc = bcast_row(g2_d, D_) if cfg["has_n2"] else None
        b2_bc = bcast_row(b2_d, D_) if cfg["has_n2"] else None

        qT_sb = [slot(f"qT{c}", T_, BF16) for c in range(KD)]
        kT_sb = [slot(f"kT{c}", T_, BF16) for c in range(KD)]
        Vp8_sb = [slot(f"Vp{c}", 2 * VROW, FP8) for c in range(TCH // 2)]

        HB = P // 2

        # ---- QKV projections (fp8 DoubleRow) ----
        def qk_proj(nm, m, t2):
            has_b = cfg["has_bq"] if nm == "q" else cfg["has_bk"]
            bias = (bq_sb if nm == "q" else bk_sb) if has_b else None
            pt = aux_tile()
            for c2 in range(KD2):
                nc.tensor.matmul(
                    pt[:, :NTW],
                    lhsT=pair(w_sb[nm][c2], slice(m * P, (m + 1) * P)),
                    rhs=pair(xT8_sb[c2], slice(t2 * MM_N, t2 * MM_N + NTW)),
                    start=(c2 == 0), stop=(c2 == KD2 - 1),
                    perf_mode=DR)
            sl = slice(t2 * MM_N, t2 * MM_N + NTW)
            dst = (qT_sb if nm == "q" else kT_sb)[m]
            if has_b:
                nc.vector.tensor_scalar_add(out=dst[:, sl], in0=pt[:, :NTW],
                                            scalar1=bias[:, m:m + 1])
            else:
                # vector only: an evict on ScalarE would head-of-line
                # block the exp stream behind this op's DMA-gated matmul
                nc.vector.tensor_copy(out=dst[:, sl], in_=pt[:, :NTW])

        # V token-major into V' ([v(64), 1] per head; 2^-7 scale on evict)
        def v_proj_tr(tr):
            vdst = Vp8_sb[tr // 2][:, (tr % 2) * VROW:(tr % 2 + 1) * VROW]
            vd3 = vdst.rearrange("p (h c) -> p h c", c=VW)
            for n2 in range(ND):
                pt = aux_tile()
                for c2 in range(KD2):
                    nc.tensor.matmul(
                        pt[:, :NDW],
                        lhsT=pair(xT8_sb[c2], slice(tr * P, (tr + 1) * P)),
                        rhs=pair(w_sb["v"][c2],
                                 slice(n2 * MM_N, n2 * MM_N + NDW)),
                        start=(c2 == 0), stop=(c2 == KD2 - 1),
                        perf_mode=DR)
                hpn = NDW // HD_   # heads per N tile
                src = pt[:, :NDW].rearrange("p (h c) -> p h c", c=HD_)
                if cfg["has_bv"]:
                    tmp = work.tile([P, NDW], F32, tag="vtmp", name="vtmp")
                    nc.vector.tensor_add(
                        out=tmp, in0=pt[:, :NDW],
                        in1=bv_bc[:, n2 * MM_N:n2 * MM_N + NDW])
                    src = tmp.rearrange("p (h c) -> p h c", c=HD_)
                nc.vector.tensor_scalar_mul(
                    out=vd3[:, n2 * hpn:(n2 + 1) * hpn, 0:HD_], in0=src,
                    scalar1=1.0 / WS)
            nc.vector.memset(vd3[:, :, HD_:VW], 1.0)  # ones cols

        # ---- attention machinery ----
        # wo loads follow wv on the bulk queues (needed mid-b1 for fills)
        wo_sb = []
        for c2 in range(KD2):
            t = slot(f"wo{c2}", 2 * D_, FP8)
            cold_load(t, wo_d[c2])
            wo_sb.append(t)
        oT8_sb = [slot(f"oT{c2}", 2 * T_, FP8) for c2 in range(KD2)]
        oTu_sb = [slot(f"oTu{hc}", T_, BF16) for hc in range(KD)]
        den_d = dramp.tile([H_, T_], F32, tag="den_d", name="den_d")
        rb_sb = {}

        def at_tile():
            return attnp.tile([P, 2 * S_], FP8, tag="attnT", bufs=10 + 2,
                              name="attnT")

        def score_tile(b, hc, par, half):
            """One [128,1024] psum tile: head-parity `par`, key chunks
            2*half/2*half+1. K=64 at base partition 0/64: the row group
            auto-derives, so the two heads' matmuls share the array."""
            pt = sc_tile()
            for k2 in range(2):
                kc = 2 * half + k2
                nc.tensor.matmul(
                    pt[:, k2 * S_:(k2 + 1) * S_],
                    lhsT=kT_sb[hc][par * HD_:(par + 1) * HD_,
                                   b * S_ + kc * P:b * S_ + (kc + 1) * P],
                    rhs=qT_sb[hc][par * HD_:(par + 1) * HD_,
                                  b * S_:(b + 1) * S_],
                    start=True, stop=True)
            return pt

        def exp_tile(b, pt, half):
            at = at_tile()
            if cfg["has_mask"]:
                for k2 in range(2):
                    kc = 2 * half + k2
                    nc.scalar.activation(
                        out=at[:, k2 * S_:(k2 + 1) * S_],
                        in_=pt[:, k2 * S_:(k2 + 1) * S_],
                        func=mybir.ActivationFunctionType.Exp,
                        bias=mask2_sb[:, b * SKC + kc:b * SKC + kc + 1],
                        scale=EXP_SCALE)
            else:
                nc.scalar.activation(
                    out=at, in_=pt[:, 0:2 * S_],
                    func=mybir.ActivationFunctionType.Exp,
                    bias=attb_t, scale=EXP_SCALE)
            return at

        def attn_av(b, hc, ats):
            for par in range(HPC):
                h = hc * HPC + par
                pv = aux_tile()
                for half in range(2):
                    nc.tensor.matmul(
                        pv[0:VW, :S_],
                        lhsT=pair(Vp8_sb[b * SKC2 + half],
                                  slice(h * VW, (h + 1) * VW)),
                        rhs=pair(ats[2 * half + par], None),
                        start=(half == 0), stop=(half == 1),
                        perf_mode=DR)
                ho = par * HD_
                nc.vector.tensor_copy(
                    out=oTu_sb[hc][ho:ho + HD_, b * S_:(b + 1) * S_],
                    in_=pv[0:HD_, :S_])
                rs = attn2.tile([1, S_], F32, tag="rs", bufs=3, name="rs")
                nc.vector.tensor_copy(out=rs, in_=pv[HD_:VW, :S_])
                dma2.dma_start(out=den_d[h:h + 1, b * S_:(b + 1) * S_],
                               in_=rs)

        def rb_load(b, hc):
            # broadcast this chunk's denominators back from DRAM
            sl = slice(b * S_, (b + 1) * S_)
            rb = rbp.tile([P, S_], F32, tag=f"rb{hc % 4}", name="rb")
            rb_sb[hc] = rb
            for h2 in range(HPC):
                dma2.dma_start(
                    out=rb[h2 * HD_:(h2 + 1) * HD_, :],
                    in_=den_d[HPC * hc + h2:HPC * hc + h2 + 1, sl]
                    .to_broadcast([HD_, S_]))

        def recip_mul(b, hc):
            # reciprocal + normalize one feature chunk: oT8 = oTu / den.
            # Runs one chunk behind rb_load so the DMA latency is hidden.
            sl = slice(b * S_, (b + 1) * S_)
            rb = rb_sb[hc]
            nc.vector.reciprocal_approx_fast(out=rb, in_=rb)
            nc.vector.tensor_mul(
                out=oT8_sb[hc // 2][:, (hc % 2) * T_ + b * S_:
                                    (hc % 2) * T_ + (b + 1) * S_],
                in0=oTu_sb[hc][:, sl], in1=rb)

        # O-proj machinery; LN1 computes rstd with a vector-side Newton
        # rsqrt (seed 2^-7: the LN1 input is 2^7-scaled, so var ~= 2^14)
        # so no ScalarE act-table switch ever interrupts the exp stream.
        xm_bf = {}
        xmT8_sb = [slot(f"xmT{c2}", 2 * T_, FP8) for c2 in range(KD2)]

        def newton_rstd(v_col, eng):
            # 1/sqrt(v) for v ~ 2^14 * [0.8, 2.0]; 3 iterations to fp32-ish
            y = statp.tile([P, 1], F32, tag="nwy", name="nwy")
            t = statp.tile([P, 1], F32, tag="nwt", name="nwt")
            eng.memset(y, 2.0 ** -7)
            for _ in range(3):
                eng.tensor_mul(out=t, in0=y, in1=y)
                eng.tensor_mul(out=t, in0=t, in1=v_col)
                eng.tensor_scalar(
                    out=t, in0=t, scalar1=-0.5, scalar2=1.5,
                    op0=mybir.AluOpType.mult, op1=mybir.AluOpType.add)
                eng.tensor_mul(out=y, in0=y, in1=t)
            return y

        def ln1_tr(tr):
            # in-place LayerNorm on the bf16 x_medium tile (vector-only;
            # gpsimd bulk elementwise is ~17x slower than DVE)
            xm = xm_bf[tr]
            bw = min(512, D_)
            nsub = _ceil_div(D_, bw)
            st = statp.tile([P, nsub, 6], F32, tag="bnst", name="bnst")
            for i in range(nsub):
                nc.vector.bn_stats(out=st[:, i, :],
                                   in_=xm[:, i * bw:(i + 1) * bw])
            mv = statp.tile([P, 2], F32, tag="bnmv", name="bnmv")
            nc.vector.bn_aggr(out=mv, in_=st)
            if tr < TCH // 2:
                # trs 0-3 run before any gelu: ScalarE sqrt is free and
                # much shorter than the 13-op Newton chain on vector
                rstd = statp.tile([P, 1], F32, tag="rstd1", name="rstd1")
                nc.scalar.activation(out=rstd, in_=mv[:, 1:2],
                                     func=mybir.ActivationFunctionType.Sqrt,
                                     bias=eps_t, scale=1.0)
                nc.vector.reciprocal(out=rstd, in_=rstd)
            else:
                rstd = newton_rstd(mv[:, 1:2], nc.vector)
            if cfg["has_n1"]:
                tmp = statp.tile([P, D_], F32, tag="lntmp", name="lntmp")
                nc.vector.tensor_scalar(
                    out=tmp, in0=xm, scalar1=mv[:, 0:1], scalar2=rstd,
                    op0=mybir.AluOpType.subtract, op1=mybir.AluOpType.mult)
                nc.vector.tensor_mul(out=tmp, in0=tmp, in1=g1_bc)
                nc.vector.tensor_add(out=xm, in0=tmp, in1=b1_bc)
            else:
                nc.vector.tensor_scalar(
                    out=xm, in0=xm, scalar1=mv[:, 0:1], scalar2=rstd,
                    op0=mybir.AluOpType.subtract, op1=mybir.AluOpType.mult)

        xr_tiles = {}

        def xr_load(tr):
            xt = xrp.tile([P, D_], BF16, tag="xrt", name="xrt")
            dma2.dma_start(out=xt, in_=xr_d[tr * P:(tr + 1) * P, :])
            xr_tiles[tr] = xt

        def o_mm_tr(tr):
            # O-projection matmuls + residual add -> bf16 xm (pre-LN)
            xt = xr_tiles[tr]
            xm = slot(f"qT{tr}", D_, BF16)   # reuse qT slot (scores done)
            xm_bf[tr] = xm
            for n2 in range(ND):
                pt = aux_tile()
                for c2 in range(KD2):
                    nc.tensor.matmul(
                        pt[:, :NDW],
                        lhsT=pair(oT8_sb[c2], slice(tr * P, (tr + 1) * P)),
                        rhs=pair(wo_sb[c2],
                                 slice(n2 * MM_N, n2 * MM_N + NDW)),
                        start=(c2 == 0), stop=(c2 == KD2 - 1),
                        perf_mode=DR)
                nc.vector.tensor_add(out=xm[:, n2 * MM_N:n2 * MM_N + NDW],
                                     in0=pt[:, :NDW],
                                     in1=xt[:, n2 * MM_N:n2 * MM_N + NDW])
                if cfg["has_bo"]:
                    nc.vector.tensor_add(
                        out=xm[:, n2 * MM_N:n2 * MM_N + NDW],
                        in0=xm[:, n2 * MM_N:n2 * MM_N + NDW],
                        in1=bo_bc[:, n2 * MM_N:n2 * MM_N + NDW])

        def o_proj_tr(tr):
            o_mm_tr(tr)
            ln1_tr(tr)

        def transpose_tr(tr):
            # PE transposes, evicted 2-at-a-time with a 3D [128,2,128] AP
            # (DVE only: gpsimd has no PSUM port)
            for c2 in range(KD2):
                pt = pall.tile([P, 2 * P], BF16, tag="aux", bufs=2,
                               name="ps_t", padded_shape=[P, MM_N])
                for j in range(2):
                    c = 2 * c2 + j
                    nc.tensor.transpose(pt[:, j * P:(j + 1) * P],
                                        xm_bf[tr][:, c * P:(c + 1) * P],
                                        ident)
                dst = xmT8_sb[c2].rearrange(
                    "p (i w) -> p i w", i=2)[:, :, tr * P:(tr + 1) * P]
                nc.vector.tensor_copy(out=dst,
                                      in_=pt.rearrange("p (i w) -> p i w",
                                                       i=2))

        # ---- FFN up helpers (t2-split halves) ----
        wup_sb = {}

        def wup_load(i, tag, cold=False):
            t = slot(tag, 2 * UPW, FP8)
            if cold:
                cold_load(t, wup_d[i])
            else:
                dma.dma_start(out=t, in_=wup_d[i])
            wup_sb[i] = t

        # the fm<16 half of wup goes into the idle dn tags NOW (trickles
        # in during b0 attention) so FFN up can start the moment the
        # attention loop ends
        for c2 in range(KD2):
            wup_load(c2 * UPT + 0, f"dn{c2}", cold=True)
        for c2 in range(KD2):
            wup_load(c2 * UPT + 1, f"dn{4 + c2}", cold=True)

        def wup_lhsT(c2, fm):
            i = c2 * UPT + (fm * P) // UPW
            o = (fm * P) % UPW
            return pair(wup_sb[i], slice(o, o + P))

        gT_sb = {}
        # tag order matters: oT tags free only after o_proj_tr(7), which is
        # woven at up-t2=0 fm==23 -> oT tags must serve fm>=28 only
        g_tags = ([f"g{c}" for c in range(KF - KD - 2 * KD2)]
                  + [f"wv{c2}" for c2 in range(KD2)]
                  + [f"oTu{hc}" for hc in range(KD)]
                  + [f"oT{c2}" for c2 in range(KD2)])

        def up_half(fm, t2):
            pt = aux_tile()
            for c2 in range(KD2):
                nc.tensor.matmul(
                    pt[:, :NTW],
                    lhsT=wup_lhsT(c2, fm),
                    rhs=pair(xmT8_sb[c2],
                             slice(t2 * MM_N, t2 * MM_N + NTW)),
                    start=(c2 == 0), stop=(c2 == KD2 - 1),
                    perf_mode=DR)
            if fm not in gT_sb:
                gT_sb[fm] = slot(g_tags[fm], T_, BF16)
            nc.scalar.activation(
                out=gT_sb[fm][:, t2 * MM_N:t2 * MM_N + NTW],
                in_=pt[:, :NTW],
                func=mybir.ActivationFunctionType.Gelu,
                bias=(bup_sb[:, fm:fm + 1] if cfg["has_bup"] else zero_t),
                scale=1.0 / WS)

        # ---- emit: Q0/K0 then the exp-stream-driven attention loop ----
        qk_proj("q", 0, 0)
        qk_proj("k", 0, 0)

        # fills per (b, pair-index): list of thunks. Constraints:
        #  - v(0..3) emitted by slot (0,1) (first AV dequeues at (0,2));
        #    v(4..7) by slot (1,1).
        #  - qk(m,0) by slot (0,m-1); qk(m,1) by slot (1,m-1) (or in b0).
        def F_qk(m, t2):
            return lambda: (qk_proj("q", m, t2), qk_proj("k", m, t2))

        def F_v(tr):
            return lambda: v_proj_tr(tr)

        fills = {
            (0, 0): [F_qk(1, 0), F_v(0), F_v(1)],
            (0, 1): [F_qk(2, 0), F_v(2), F_v(3)],
            (0, 2): [F_qk(3, 0)], (0, 3): [F_qk(4, 0)],
            (0, 4): [F_qk(5, 0)], (0, 5): [F_qk(6, 0)],
            (0, 6): [F_qk(7, 0), F_qk(0, 1)],
            (0, 7): [F_qk(1, 1), F_v(4)],
            (1, 0): [F_v(5), F_qk(2, 1)],
            (1, 1): [F_v(6), F_v(7), F_qk(3, 1)],
            (1, 2): [F_qk(4, 1)],
            (1, 3): [F_qk(5, 1)],
            (1, 4): [F_qk(6, 1)],
            (1, 5): [F_qk(7, 1)],
        }

        avq = []   # 2-deep AV lag so V fills land before the first AV
        rmq = []   # recip_mul runs one chunk behind rb_load

        def av_dequeue():
            pb, phc, ats = avq.pop(0)
            attn_av(pb, phc, ats)
            rb_load(pb, phc)
            rmq.append((pb, phc))
            if len(rmq) >= 2:
                recip_mul(*rmq.pop(0))

        for b in range(TB_):
            for hc in range(KD):
                # A/B/C tiles + exps, then AV of pair-2 (PE work bridging
                # the D tile's psum-tag WAR wait on expA), then D
                ptA = score_tile(b, hc, 0, 0)
                ptB = score_tile(b, hc, 1, 0)
                ptC = score_tile(b, hc, 0, 1)
                atA = exp_tile(b, ptA, 0)
                atB = exp_tile(b, ptB, 0)
                atC = exp_tile(b, ptC, 1)
                if len(avq) >= 2:
                    av_dequeue()
                ptD = score_tile(b, hc, 1, 1)
                atD = exp_tile(b, ptD, 1)
                avq.append((b, hc, [atA, atB, atC, atD]))
                for f in fills.get((b, hc), []):
                    f()
        while avq:
            pb, phc, ats = avq.pop(0)
            attn_av(pb, phc, ats)
            rb_load(pb, phc)
            rmq.append((pb, phc))
            if len(rmq) > 2:
                recip_mul(*rmq.pop(0))
        # remaining recip_muls (b1 tail) are deferred: they're only needed
        # by o_proj_tr(4..7), and emitting them here would put ~5us of
        # vector work in front of the O-phase adds/LN1 chains

        # ---- post-attention ----
        # wup fm>=16 half: j=2 blocks into wq tags (free after the
        # deferred QK fills), j=3 into wk tags.
        for tr in range(TCH):
            xr_load(tr)
        for c2 in range(KD2):
            wup_load(c2 * UPT + 2, f"wq{c2}")
        for c2 in range(KD2):
            wup_load(c2 * UPT + 3, f"wk{c2}")
        # O-proj/LN1/transpose of trs 0-3 unlock up-t2=0 (its xmT8 column
        # slice only spans batch-0 tokens)
        o_proj_tr(0)
        o_proj_tr(1)
        transpose_tr(0)
        o_proj_tr(2)
        transpose_tr(1)
        o_proj_tr(3)
        transpose_tr(2)
        transpose_tr(3)
        while rmq:
            recip_mul(*rmq.pop(0))

        # FFN down weight loads issued early so the DMA hides under up
        # wo tags free only after o_proj_tr(7): keep them last so the sync
        # queue isn't blocked mid-stream waiting on the weave
        dn_tags = ([f"Vp{c}" for c in range(TCH // 2)]
                   + [f"xT8{c2}" for c2 in range(KD2)]
                   + [f"kT{c}" for c in range(KD)]
                   + [f"dn{i}" for i in range(8)]
                   + [f"wo{c2}" for c2 in range(KD2)])
        wdn_sb = []
        for fc in range(KF):
            if fc < len(dn_tags):
                t = slot(dn_tags[fc], D_, BF16)
            else:
                t = rbp.tile([P, D_], BF16, tag=f"rb{fc - len(dn_tags)}",
                             name="t_wdn_rb")
            # sync queue only: scalar must stay free for the gelu stream
            dma.dma_start(out=t, in_=wdn_d[fc * P:(fc + 1) * P, :])
            wdn_sb.append(t)

        # up-t2=0 matmuls hide the vector-bound O/LN1 chains of trs 4-7
        weave = {2: lambda: o_proj_tr(4), 7: lambda: transpose_tr(4),
                 9: lambda: o_proj_tr(5), 14: lambda: transpose_tr(5),
                 16: lambda: o_proj_tr(6), 21: lambda: transpose_tr(6),
                 23: lambda: o_proj_tr(7), 28: lambda: transpose_tr(7)}
        for fm in range(KF):
            up_half(fm, 0)
            if fm in weave:
                weave[fm]()
        for fm in range(KF):
            up_half(fm, 1)

        # ---- FFN down (bf16) + residual + LN2 -> out, incremental ----
        def layer_norm_apply(src_t, dst, mv, rstd, g_bc, b_bc):
            if g_bc is None:
                nc.vector.tensor_scalar(
                    out=dst, in0=src_t, scalar1=mv[:, 0:1], scalar2=rstd,
                    op0=mybir.AluOpType.subtract, op1=mybir.AluOpType.mult)
            else:
                tmp = statp.tile([P, D_], F32, tag="lntmp", name="lntmp")
                nc.vector.tensor_scalar(
                    out=tmp, in0=src_t, scalar1=mv[:, 0:1], scalar2=rstd,
                    op0=mybir.AluOpType.subtract, op1=mybir.AluOpType.mult)
                nc.vector.tensor_mul(out=tmp, in0=tmp, in1=g_bc)
                nc.vector.tensor_add(out=dst, in0=tmp, in1=b_bc)

        for tr in range(TCH):
            dsb = work.tile([P, D_], F32, tag="acc", name="dsb")
            st = statp.tile([P, ND, 6], F32, tag="bnst", name="bnst")
            pt = sc_tile()
            for n2 in range(ND):
                for fc in range(KF):
                    nc.tensor.matmul(
                        pt[:, n2 * MM_N:n2 * MM_N + NDW],
                        lhsT=gT_sb[fc][:, tr * P:(tr + 1) * P],
                        rhs=wdn_sb[fc][:, n2 * MM_N:n2 * MM_N + NDW],
                        start=(fc == 0), stop=(fc == KF - 1))
                # evict+add+stats per half so only the last half's chain
                # is exposed after the final matmul
                sl = slice(n2 * MM_N, n2 * MM_N + NDW)
                nc.vector.tensor_add(out=dsb[:, sl], in0=pt[:, sl],
                                     in1=xm_bf[tr][:, sl])
                if cfg["has_bdn"]:
                    nc.vector.tensor_add(out=dsb[:, sl], in0=dsb[:, sl],
                                         in1=bdn_bc[:, sl])
                nc.vector.bn_stats(out=st[:, n2, :], in_=dsb[:, sl])
            mv = statp.tile([P, 2], F32, tag="bnmv", name="bnmv")
            nc.vector.bn_aggr(out=mv, in_=st)
            rstd = statp.tile([P, 1], F32, tag="rstd", name="rstd")
            nc.scalar.activation(out=rstd, in_=mv[:, 1:2],
                                 func=mybir.ActivationFunctionType.Sqrt,
                                 bias=eps_t, scale=1.0)
            nc.vector.reciprocal(out=rstd, in_=rstd)
            ot = outp.tile([P, D_], F32, tag="ot", name="ot")
            layer_norm_apply(dsb, ot, mv, rstd,
                             g2_bc if cfg["has_n2"] else None,
                             b2_bc if cfg["has_n2"] else None)
            if tr < TCH - 1:
                dma.dma_start(out=out_d[tr * P:(tr + 1) * P, :], in_=ot)
            else:
                # last chunk is latency-exposed: split across HW queues
                qw = D_ // 4
                engs = (nc.sync, nc.scalar, nc.sync, nc.scalar)
                for qi, eng in enumerate(engs):
                    eng.dma_start(
                        out=out_d[tr * P:(tr + 1) * P,
                                  qi * qw:(qi + 1) * qw],
                        in_=ot[:, qi * qw:(qi + 1) * qw])

    nc.finalize()
    return nc


_PROGRAM_CACHE = {}


def _get_program(cfg_key, cfg):
    if cfg_key not in _PROGRAM_CACHE:
        _PROGRAM_CACHE[cfg_key] = build_program(cfg)
    return _PROGRAM_CACHE[cfg_key]


def _swz(w, npairs, width):
    """[rows, cols] -> [npairs, 128, 2*cols] K-paired contiguous."""
    return np.ascontiguousarray(
        w.reshape(npairs, 2, P, width).transpose(0, 2, 1, 3)
        .reshape(npairs, P, 2 * width))


def make_in_maps(inputs):
    f32 = np.float32
    x = np.asarray(inputs["x"], f32)
    scale = 1.0 / np.sqrt(float(inputs["head_dim"]))

    def merged(w, a, b):
        return (np.asarray(w, f32)
                + np.asarray(a, f32) @ np.asarray(b, f32))

    KD2 = D // P // 2
    wq = _swz((merged(inputs["w_q"], inputs["w_q_lora_a"],
                      inputs["w_q_lora_b"]) * (scale * QS)).astype(NP_FP8),
              KD2, D)
    wk = _swz((merged(inputs["w_k"], inputs["w_k_lora_a"],
                      inputs["w_k_lora_b"]) * WS).astype(NP_FP8), KD2, D)
    wv = _swz((merged(inputs["w_v"], inputs["w_v_lora_a"],
                      inputs["w_v_lora_b"]) * WS).astype(NP_FP8), KD2, D)
    wo = _swz((merged(inputs["w_o"], inputs["w_o_lora_a"],
                      inputs["w_o_lora_b"]) * WS).astype(NP_FP8), KD2, D)
    wup8 = (merged(inputs["w_up"], inputs["w_up_lora_a"],
                   inputs["w_up_lora_b"]) * WS).astype(NP_FP8)
    UPW = 1024
    UPT = F // UPW
    wup = np.ascontiguousarray(
        wup8.reshape(KD2, 2, P, UPT, UPW).transpose(0, 3, 2, 1, 4)
        .reshape(KD2 * UPT, P, 2 * UPW))
    wdn = merged(inputs["w_down"], inputs["w_down_lora_a"],
                 inputs["w_down_lora_b"]).astype(NP_BF16)
    mask = np.asarray(inputs["attention_mask"], f32)

    common = {
        "wq": wq, "wk": wk, "wv": wv, "wo": wo, "wup": wup, "wdn": wdn,
        "bq": (np.asarray(inputs["b_q"], f32) * (scale * QS)).astype(f32),
        "bk": (np.asarray(inputs["b_k"], f32) * WS).astype(f32),
        "bup": np.asarray(inputs["b_up"], f32),
        "bv": np.asarray(inputs["b_v"], f32),
        "bo": np.asarray(inputs["b_o"], f32),
        "bdn": np.asarray(inputs["b_down"], f32),
        "g1": np.asarray(inputs["norm_weight_1"], f32),
        "b1": np.asarray(inputs["norm_bias_1"], f32),
        "g2": np.asarray(inputs["norm_weight_2"], f32),
        "b2": np.asarray(inputs["norm_bias_2"], f32),
    }
    in_maps = []
    for i in range(N_CORES):
        xc = x[i * TB:(i + 1) * TB].reshape(T, D)
        m = dict(common)
        m["xT8"] = _swz(np.ascontiguousarray(xc.T).astype(NP_FP8), KD2, T)
        m["xr"] = (np.ascontiguousarray(xc) * WS).astype(NP_BF16)
        m["maskT"] = np.ascontiguousarray(mask[i * TB:(i + 1) * TB, 0, 0, :])
        in_maps.append(m)
    return in_maps


def full_cfg(inputs):
    f32 = np.float32
    return {
        "D": D, "F": F, "T": T, "TB": TB, "H": H, "HD": HD,
        "has_bq": bool(np.any(np.asarray(inputs["b_q"], f32))),
        "has_bk": bool(np.any(np.asarray(inputs["b_k"], f32))),
        "has_bup": bool(np.any(np.asarray(inputs["b_up"], f32))),
        "has_mask": bool(np.any(np.asarray(inputs["attention_mask"], f32))),
        "has_bv": bool(np.any(np.asarray(inputs["b_v"], f32))),
        "has_bo": bool(np.any(np.asarray(inputs["b_o"], f32))),
        "has_bdn": bool(np.any(np.asarray(inputs["b_down"], f32))),
        "has_n1": bool(np.any(np.asarray(inputs["norm_weight_1"], f32) != 1.0)
                       or np.any(np.asarray(inputs["norm_bias_1"], f32))),
        "has_n2": bool(np.any(np.asarray(inputs["norm_weight_2"], f32) != 1.0)
                       or np.any(np.asarray(inputs["norm_bias_2"], f32))),
    }


def run_on_hw(inputs, trace=False, tmpdir=None):
    cfg = full_cfg(inputs)
    cfg_key = tuple(sorted((k, v) for k, v in cfg.items()
                           if not isinstance(v, set)))
    nc = _get_program(cfg_key, cfg)
    in_maps = make_in_maps(inputs)
    kw = {}
    if trace:
        kw = {"trace": True, "tmpdir": tmpdir}
    res = run_bass_kernel_spmd(nc, in_maps, core_ids=list(range(N_CORES)),
                               **kw)
    out = np.empty((B, S, D), np.float32)
    for i in range(N_CORES):
        out[i * TB:(i + 1) * TB] = res.results[i]["out"].reshape(TB, S, D)
    return out, res


def kernel(**inputs):
    out, _ = run_on_hw(inputs)
    return out
